# revision 49
# baseline (speedup 1.0000x reference)
"""Trainium2 Bass kernel for the LoRA-QKV + per-frame local attention +
cross-frame CLS attention + adapter module (nn_Attention sparse_attention).

Contract: kernel(**inputs) takes FULL unsharded inputs (as in
reference.setup_inputs()), shards the video batch over 8 NeuronCores
(2 videos = 24 frames per core), runs one SPMD Bass program, and returns
the FULL [192, 197, 768] fp32 output.

Math notes (exact algebra, not approximations):
  - qkv = x@(W + lora_b@lora_a).T + in_proj_bias  (LoRA folded on host)
  - v bias is folded through the out projection: attn@(v + 1 b_v^T) @ Wo^T
    = attn@v @ Wo^T + 1 (b_v @ Wo^T)^T, merged with out_proj_bias into one
    rank-1 bias row added via a K=1 matmul.
  - softmax computed without max subtraction (scores here are O(1); exp is
    well inside fp32 range), matching softmax exactly in exact arithmetic.
  - cross-frame attention outputs are normalized after the AV matmul
    (linearity of AV in the attention weights).
Matmuls run in bf16 with fp32 PSUM accumulation.

Performance structure (v3, 612.7us vs 1063us baseline on TRN2):
  - x is pre-transposed and pre-cast to bf16 on the host, fed as per-pair
    tiles xT [128, 6*394] (two seqs side by side); QKV weights are packed
    m-block-major so the first matmul only waits on a 192KB DMA.
  - QKV runs 2 seqs per matmul (N=394) with the weight tile stationary;
    merged exp covers both key chunks of a head in one activation (the
    PSUM zero-region rows are defined via persistent ring tiles).
  - emission is a software-pipelined task queue: per-head attention work of
    pair p-1 (scores -> exp -> AV -> normalize) is interleaved between the
    QKV/V-proj chunks of pair p, with every dependent tensor op placed >=2
    tasks behind its scalar/vector producer so the in-order tensor queue
    never head-blocks (keeps the PE near its top p-state).
  - k^T tiles stay SBUF-resident for the whole video; the cross-frame
    epilogue computes scores transposed ([keys, queries]) so the attention
    probabilities feed the AV matmul directly -- no PE transposes or
    PSUM->SBUF repacks -- with softmax sums from ones-columns in the spilled
    v tiles and normalization applied at extraction.
  - softmax reciprocal rows are broadcast to 64 partitions with the gpsimd
    partition_broadcast DMA (input must be at partition 0 on hardware).
  - v spills / reloads are gpsimd-issued DMAs (25ns dispatch vs ~700ns on
    sync), contiguous in a [keys, 780] layout.
"""

import sys
from collections import deque

sys.path.insert(0, "/opt/trn_rl_repo")

import numpy as np
import ml_dtypes

import concourse.bass as bass
import concourse.mybir as mybir
import concourse.tile as tile
from concourse import bacc
from concourse.bass_utils import run_bass_kernel_spmd
from concourse.masks import make_identity

F32 = mybir.dt.float32
BF16 = mybir.dt.bfloat16
AF = mybir.ActivationFunctionType
MUL = mybir.AluOpType.mult

NCORES = 8
B, F, T, E, H, D, R = 16, 12, 197, 768, 12, 64, 8
NV = B // NCORES          # videos per core = 2
S = NV * F                # seqs per core = 24
NP = S // 2               # seq pairs per core = 12
KT = E // 128             # 6 feature k-tiles
T2 = 2 * T                # 394: two seqs of tokens side by side
G = 8                     # seqs per out-proj group
NG = S // G
GT = G * T                # tokens per group = 1576
SCALE = float(D) ** -0.5
TQ = T + 1                # 198: pair column stride inside ps_o (4B-aligned)
FT = F * T                # keys per video for cross-frame = 2364
NKC = 2 * F               # irregular key chunks per video (128/69 per frame)

_last_results = None  # test harness reads exec_time_ns from here


def _bf(x):
    return np.ascontiguousarray(x.astype(ml_dtypes.bfloat16))


def _f32(x):
    return np.ascontiguousarray(x.astype(np.float32))


def _build(has_qk_bias, has_orow_bias, has_down_bias, has_cls_bias):
    nc = bacc.Bacc("TRN2", target_bir_lowering=False, debug=False,
                   num_devices=NCORES)

    xt_d = nc.declare_dram_parameter("xt", [NP, 128, KT * T2], BF16,
                                     isOutput=False)
    wqk_d = nc.declare_dram_parameter("w_qkt", [2 * KT, 128, KT * 128], BF16,
                                      isOutput=False)
    wv_d = nc.declare_dram_parameter("w_vt", [128, KT * E], BF16,
                                     isOutput=False)
    wo_d = nc.declare_dram_parameter("w_ot", [128, KT * E], BF16,
                                     isOutput=False)
    bqk_d = nc.declare_dram_parameter("b_qk_t", [128, 2 * KT], F32,
                                      isOutput=False)
    brow_d = nc.declare_dram_parameter("bias_row_o", [1, E], BF16,
                                       isOutput=False)
    bcls_d = nc.declare_dram_parameter("b_cls_t", [128, KT], F32,
                                       isOutput=False)
    dwt_d = nc.declare_dram_parameter("down_wt", [128, KT * R], BF16,
                                      isOutput=False)
    bdown_d = nc.declare_dram_parameter("b_down", [R, 1], F32, isOutput=False)
    uwt_d = nc.declare_dram_parameter("up_wt", [R, E], BF16, isOutput=False)

    y_d = nc.declare_dram_parameter("y", [S, T, E], F32, isOutput=True)
    vs_d = nc.dram_tensor("v_scr", [NV, FT, H * (D + 1)], BF16)

    y_flat = y_d.ap().rearrange("a b c -> (a b) c")

    with tile.TileContext(nc) as tc:
        with (
            tc.tile_pool(name="cst", bufs=1) as cst,
            tc.tile_pool(name="sb", bufs=2) as sb,
            tc.tile_pool(name="psmm", bufs=2, space="PSUM") as psmm,
            tc.tile_pool(name="pssc", bufs=1, space="PSUM") as pssc,
            tc.tile_pool(name="psav", bufs=3, space="PSUM") as psav,
            tc.tile_pool(name="psep", bufs=1, space="PSUM") as psep,
        ):
            # ---------------- input prefetch + constants ----------------
            # first two xt pair-tiles go out before the weights so the QKV
            # pipeline can start as soon as wqk lands
            xt_tiles = {}

            def load_xt(p):
                t_ = sb.tile([128, KT * T2], BF16, tag="xt", bufs=4,
                             name=f"xt{p}")
                nc.sync.dma_start(out=t_[:], in_=xt_d[p])
                xt_tiles[p] = t_

            # startup DMAs: three independent issue queues, ordered by
            # first-use time (queues sustain ~90GB/s each; the q-half of the
            # QKV weights gates the first matmuls, wvt gates the first
            # V-projection, the k-half lands on the scalar queue in parallel)
            wvt = cst.tile([128, KT * E], BF16, tag="wvt")
            nc.gpsimd.dma_start(out=wvt[:], in_=wv_d[:, :])
            load_xt(0)
            wqkm = []
            for m in range(2 * KT):
                t_ = cst.tile([128, KT * 128], BF16, tag=f"wqkm{m}",
                              name=f"wqkm{m}")
                eng = nc.sync if m < KT else nc.scalar
                eng.dma_start(out=t_[:], in_=wqk_d[m])
                wqkm.append(t_)
            load_xt(1)
            wot = cst.tile([128, KT * E], BF16, tag="wot")
            nc.scalar.dma_start(out=wot[:], in_=wo_d[:, :])
            bqk = cst.tile([128, 2 * KT], F32, tag="bqk")
            nc.gpsimd.dma_start(out=bqk[:], in_=bqk_d[:, :])
            brow = cst.tile([1, E], BF16, tag="brow")
            nc.gpsimd.dma_start(out=brow[:], in_=brow_d[:, :])
            bcls = cst.tile([128, KT], F32, tag="bcls")
            nc.gpsimd.dma_start(out=bcls[:], in_=bcls_d[:, :])
            dwtt = cst.tile([128, KT * R], BF16, tag="dwtt")
            nc.gpsimd.dma_start(out=dwtt[:], in_=dwt_d[:, :])
            bdown = cst.tile([R, 1], F32, tag="bdown")
            nc.gpsimd.dma_start(out=bdown[:], in_=bdown_d[:, :])
            uwt = cst.tile([R, E], BF16, tag="uwt")
            nc.gpsimd.dma_start(out=uwt[:], in_=uwt_d[:, :])

            identh = cst.tile([128, 128], BF16, tag="identh")
            make_identity(nc, identh[:])
            identf = cst.tile([128, 128], F32, tag="identf")
            make_identity(nc, identf[:])
            ones_h = cst.tile([97, 128], BF16, tag="ones_h")
            nc.vector.memset(ones_h[:], 1.0)

            # persistent cross-frame state
            q1s = [cst.tile([128, S], BF16, tag=f"q1s{m}", name=f"q1s{m}")
                   for m in range(KT)]
            ocfT = [cst.tile([128, S], BF16, tag=f"ocfT{k}", name=f"ocfT{k}")
                    for k in range(KT)]
            qbd = [[None] * KT for _ in range(NV)]

            # Persistent ring buffers: tiles whose pad regions must stay
            # defined across reuses. Using one tile id per slot keeps the
            # race detector's happens-before tracking intact (subtile deps)
            # while the one-time initialization of the pad bytes persists.
            vsl_ring = []
            for i in range(6):
                t_ = cst.tile([128, H * (D + 1)], BF16, tag=f"vsl{i}",
                              name=f"vsl{i}")
                t3 = t_[:].rearrange("p (h d) -> p h d", h=H)
                nc.gpsimd.memset(t3[:, :, D:D + 1], 1.0)
                vsl_ring.append(t_)
            # two persistent PSUM score banks: the merged exp's zero-region
            # rows read bytes only this one-time memset wrote (exp(0)=1,
            # never consumed downstream)
            psc_ring = []
            for i in range(2):
                t_ = pssc.tile([128, 512], F32, tag=f"psc{i}", name=f"psc{i}")
                nc.vector.memset(t_[:], 0.0)
                psc_ring.append(t_)
            cnt = {"psc": 0, "vsl": 0, "smt": 0}

            def next_psc():
                t_ = psc_ring[cnt["psc"] % 2]
                cnt["psc"] += 1
                return t_

            # ---------------- task machinery ----------------
            pend = deque()

            def drain(n):
                for _ in range(n):
                    if pend:
                        pend.popleft()()

            qtiles = {}     # (p, m 0..5) -> q^T tile [128, T2]
            ktiles = {}     # (p, j 0..5) -> k^T tile [128, T2]
            vslabs = {}     # (s, ci) -> vslab tile
            pT_store = {}
            smt_cur = {}
            pair_ctx = {}
            attnTg = {}
            epi_acc = {}
            epi_rcs = {}
            epi_vt = {}
            epi_pl = {}

            # ---------------- per-pair inline emission ----------------
            def emit_pair_qkv(p):
                xt = xt_tiles[p]
                for m in range(2 * KT):
                    ps = psmm.tile([128, 512], F32, tag="pmm", name="psqkv")
                    for k in range(KT):
                        nc.tensor.matmul(ps[:, :T2],
                                         wqkm[m][:, 128 * k:128 * (k + 1)],
                                         xt[:, T2 * k:T2 * (k + 1)],
                                         start=(k == 0), stop=(k == KT - 1))
                    if m < KT:
                        t_ = sb.tile([128, T2], BF16, tag="qt", bufs=12,
                                     name=f"qt{p}_{m}")
                    else:
                        t_ = sb.tile([128, T2], BF16, tag="kt", bufs=54,
                                     name=f"kt{p}_{m - KT}")
                    on_scalar = (m % 3 == 0)
                    if has_qk_bias:
                        if on_scalar:
                            nc.scalar.activation(t_[:], ps[:, :T2], AF.Identity,
                                                 bias=bqk[:, m:m + 1])
                        else:
                            nc.vector.tensor_scalar_add(t_[:], ps[:, :T2],
                                                        bqk[:, m:m + 1])
                    else:
                        if on_scalar:
                            nc.scalar.copy(t_[:], ps[:, :T2])
                        else:
                            nc.vector.tensor_copy(t_[:], ps[:, :T2])
                    if m < KT:
                        qtiles[(p, m)] = t_
                        # CLS queries: col 0 of each seq half
                        src = t_[:].rearrange("p (b c) -> p b c", c=T)[:, :, 0:1]
                        nc.gpsimd.tensor_copy(
                            q1s[m][:, 2 * p:2 * p + 2],
                            src.rearrange("p b c -> p (b c)"))
                    else:
                        ktiles[(p, m - KT)] = t_
                    drain(4)

            def emit_pair_v(p):
                xt = xt_tiles[p]
                for s in (2 * p, 2 * p + 1):
                    v, f = s // F, s % F
                    for ci in range(2):
                        off = 197 * (s % 2) + 128 * ci
                        rows = 128 if ci == 0 else 69
                        psA = psmm.tile([128, 512], F32, tag="pmm", name="psva")
                        psB = psmm.tile([128, 512], F32, tag="pmm", name="psvb")
                        for k in range(KT):
                            lh = xt[:, T2 * k + off:T2 * k + off + rows]
                            nc.tensor.matmul(psA[:rows, :512], lh,
                                             wvt[:, E * k:E * k + 512],
                                             start=(k == 0), stop=(k == KT - 1))
                            nc.tensor.matmul(psB[:rows, :256], lh,
                                             wvt[:, E * k + 512:E * (k + 1)],
                                             start=(k == 0), stop=(k == KT - 1))
                        vt = vsl_ring[cnt["vsl"] % 6]
                        cnt["vsl"] += 1
                        vt3 = vt[:].rearrange("p (h d) -> p h d", h=H)
                        # ones columns persist from the pre-init pass
                        srcA = psA[:rows, :512].rearrange("p (h d) -> p h d",
                                                          h=8)
                        srcB = psB[:rows, :256].rearrange("p (h d) -> p h d",
                                                          h=4)
                        if ci == 0:
                            nc.vector.tensor_copy(vt3[:rows, 0:8, 0:D], srcA)
                            nc.scalar.copy(vt3[:rows, 8:12, 0:D], srcB)
                        else:
                            nc.scalar.copy(vt3[:rows, 0:8, 0:D], srcA)
                            nc.vector.tensor_copy(vt3[:rows, 8:12, 0:D], srcB)
                        nc.gpsimd.dma_start(
                            out=vs_d[v, T * f + 128 * ci:T * f + 128 * ci + rows, :],
                            in_=vt[:rows, :])
                        vslabs[(s, ci)] = vt
                        drain(4)

            # ---------------- local attention tasks ----------------
            def task_sc(s, j, i):
                def go():
                    p, sc = s // 2, 197 * (s % 2)
                    kt_, qt_ = ktiles[(p, j)], qtiles[(p, j)]
                    r0 = 64 * i
                    ps_s = next_psc()
                    nc.tensor.matmul(ps_s[:, 0:T], kt_[r0:r0 + 64, sc:sc + 128],
                                     qt_[r0:r0 + 64, sc:sc + T],
                                     start=True, stop=True)
                    nc.tensor.matmul(ps_s[0:69, T:T2],
                                     kt_[r0:r0 + 64, sc + 128:sc + T],
                                     qt_[r0:r0 + 64, sc:sc + T],
                                     start=True, stop=True)
                    pT = sb.tile([128, T2], BF16, tag="pT", bufs=8,
                                 name=f"pT{s}_{j}_{i}")
                    # rows 69:128 of cols T:T2 are the bank's zero region:
                    # exp(0)=1, never read by the K=69 AV matmul
                    nc.scalar.activation(pT[:], ps_s[:, 0:T2], AF.Exp,
                                         scale=SCALE)
                    pT_store[(s, j, i)] = pT
                return go

            def task_av(s, j):
                def go():
                    g, sg = s // G, s % G
                    ps_o = psav.tile([128, 512], F32, tag="pav", name="ps_o")
                    for i in range(2):
                        h = 2 * j + i
                        pT = pT_store.pop((s, j, i))
                        vs0, vs1 = vslabs[(s, 0)], vslabs[(s, 1)]
                        nc.tensor.matmul(ps_o[:D + 1, TQ * i:TQ * i + T],
                                         vs0[:, (D + 1) * h:(D + 1) * (h + 1)],
                                         pT[:, 0:T], start=True, stop=False)
                        nc.tensor.matmul(ps_o[:D + 1, TQ * i:TQ * i + T],
                                         vs1[0:69, (D + 1) * h:(D + 1) * (h + 1)],
                                         pT[0:69, T:T2], start=False, stop=True)
                    if j % 2 == 0:
                        smt_cur[s] = sb.tile([1, 2 * T2], F32, tag="smt",
                                             bufs=3, name="smt")
                        pair_ctx[s] = []
                    smt = smt_cur[s]
                    base = T2 * (j % 2)
                    src = ps_o[D:D + 1, 0:2 * TQ].rearrange(
                        "p (b c) -> p b c", c=TQ)[:, :, 0:T]
                    dst = smt[0:1, base:base + T2].rearrange(
                        "p (b c) -> p b c", c=T)
                    if j % 2 == 0:
                        nc.scalar.copy(dst, src)
                    else:
                        nc.vector.tensor_copy(dst, src)
                    pair_ctx[s].append((j, base, ps_o))
                return go

            def task_fin(s):
                def go():
                    g, sg = s // G, s % G
                    gt = attnTg[g]
                    smt = smt_cur[s]
                    entries = pair_ctx[s]
                    rinv = sb.tile([1, 2 * T2], F32, tag="rinv", bufs=2,
                                   name="rinv")
                    nc.vector.reciprocal_approx_fast(rinv[:], smt[:])
                    for (jj, base, ps_o) in entries:
                        rb = sb.tile([D, T2], F32, tag="rb", bufs=3,
                                     name="rb")
                        nc.gpsimd.partition_broadcast(
                            rb[:], rinv[0:1, base:base + T2])
                        for i in range(2):
                            nc.vector.tensor_tensor(
                                out=gt[jj][64 * i:64 * i + 64,
                                           T * sg:T * (sg + 1)],
                                in0=ps_o[0:D, TQ * i:TQ * i + T],
                                in1=rb[:, T * i:T * (i + 1)],
                                op=MUL)
                    pair_ctx[s] = []
                return go

            def attn_tasks(p):
                out = []
                for s in (2 * p, 2 * p + 1):
                    for j in range(H // 2):
                        out.append(task_sc(s, j, 0))
                        out.append(task_sc(s, j, 1))
                        if j >= 1:
                            out.append(task_av(s, j - 1))
                            if j % 2 == 0:
                                out.append(task_fin(s))
                    out.append(task_av(s, H // 2 - 1))
                    out.append(task_fin(s))
                return out

            # ---------------- out-proj tasks ----------------
            def task_group_tile(g, tt):
                def go():
                    gt = attnTg[g]
                    c0 = 128 * tt
                    rows = min(128, GT - c0)
                    psA = psmm.tile([128, 512], F32, tag="pmm", name="psoa")
                    psB = psmm.tile([128, 512], F32, tag="pmm", name="psob")
                    laststop = not has_orow_bias
                    for k in range(KT):
                        lh = gt[k][:, c0:c0 + rows]
                        nc.tensor.matmul(psA[:rows, :512], lh,
                                         wot[:, E * k:E * k + 512],
                                         start=(k == 0),
                                         stop=(k == KT - 1 and laststop))
                        nc.tensor.matmul(psB[:rows, :256], lh,
                                         wot[:, E * k + 512:E * (k + 1)],
                                         start=(k == 0),
                                         stop=(k == KT - 1 and laststop))
                    if has_orow_bias:
                        nc.tensor.matmul(psA[:rows, :512], ones_h[:, :rows],
                                         brow[:, 0:512], start=False, stop=True)
                        nc.tensor.matmul(psB[:rows, :256], ones_h[:, :rows],
                                         brow[:, 512:768], start=False,
                                         stop=True)
                    of = sb.tile([128, E], F32, tag="of", bufs=2, name="of")
                    nc.scalar.copy(of[:rows, 0:512], psA[:rows, :512])
                    nc.vector.tensor_copy(of[:rows, 512:768], psB[:rows, :256])
                    r0 = GT * g + c0
                    nc.sync.dma_start(out=y_flat[r0:r0 + rows, :],
                                      in_=of[:rows, :])
                return go

            # ---------------- cross-frame epilogue tasks ----------------
            # Transposed formulation: scores land as [keys, queries] so the
            # attention probabilities feed the AV matmul directly as the
            # stationary operand (no PE transposes / PSUM->SBUF repacks).
            # Three passes of 4 heads (queries at 32-aligned row blocks);
            # softmax sums come from ones-columns in the v tiles and the
            # normalization is applied at extraction (AV is linear in the
            # attention weights).
            def task_qbd(v):
                def go():
                    for m in range(KT):
                        bd = cst.tile([128, 64], BF16, tag=f"qbd{v}_{m}",
                                      name=f"qbd{v}_{m}")
                        nc.gpsimd.memset(bd[:], 0.0)
                        nc.gpsimd.tensor_copy(bd[0:64, 0:F],
                                              q1s[m][0:64, F * v:F * (v + 1)])
                        nc.gpsimd.tensor_copy(bd[64:128, 32:32 + F],
                                              q1s[m][64:128, F * v:F * (v + 1)])
                        qbd[v][m] = bd
                    _issue_vt(v, 0, 0)
                    _issue_vt(v, 0, 1)
                return go

            def _chunk_geo(c):
                f2, ci = c // 2, c % 2
                return (T * f2 + 128 * ci, 128 if ci == 0 else 69, f2, ci)

            def _issue_vt(v, pp, c):
                c0, rows, f2, ci = _chunk_geo(c)
                vt_ = sb.tile([128, 4 * (D + 1)], BF16, tag="vte", bufs=8,
                              name="vte")
                nc.gpsimd.dma_start(out=vt_[:rows, :],
                                    in_=vs_d[v, c0:c0 + rows,
                                             4 * (D + 1) * pp:
                                             4 * (D + 1) * (pp + 1)])
                epi_vt[(v, pp, c)] = vt_

            def task_epi_sc(v, pp, c):
                def go():
                    c0, rows, f2, ci = _chunk_geo(c)
                    if c + 2 < NKC:
                        _issue_vt(v, pp, c + 2)
                    elif pp < 2:
                        _issue_vt(v, pp + 1, c + 2 - NKC)
                    if c == 0:
                        epi_acc[v] = psep.tile([128, 4 * (D + 1)], F32,
                                               tag="peav", name="peav")
                    ps = next_psc()
                    kt_off = 197 * (f2 % 2) + 128 * ci
                    for pl in range(2):
                        jp = 2 * pp + pl
                        kt_ = ktiles[(6 * v + f2 // 2, jp)]
                        nc.tensor.matmul(
                            ps[:rows, 64 * pl:64 * pl + 64],
                            kt_[:, kt_off:kt_off + rows], qbd[v][jp][:],
                            start=True, stop=True)
                    pl_t = sb.tile([128, 128], BF16, tag="plT", bufs=8,
                                   name="plT")
                    nc.scalar.activation(pl_t[:rows, :], ps[:rows, 0:128],
                                         AF.Exp, scale=SCALE)
                    epi_pl[(v, pp, c)] = pl_t
                return go

            def task_epi_av(v, pp, c):
                def go():
                    c0, rows, f2, ci = _chunk_geo(c)
                    pl_t = epi_pl.pop((v, pp, c))
                    vt_ = epi_vt.pop((v, pp, c))
                    nc.tensor.matmul(epi_acc[v][0:128, 0:4 * (D + 1)],
                                     pl_t[:rows, :], vt_[:rows, :],
                                     start=(c == 0), stop=(c == NKC - 1))
                return go

            def task_esum(v, pp):
                def go():
                    acc = epi_acc[v]
                    sg_ = sb.tile([128, 4], F32, tag="esum", bufs=2,
                                  name="esum")
                    nc.vector.tensor_copy(
                        sg_[:], acc[:, :].rearrange("p (h d) -> p h d",
                                                    h=4)[:, :, D:D + 1]
                        .rearrange("p h d -> p (h d)"))
                    rcs = sb.tile([128, 4], F32, tag="ercs", bufs=2,
                                  name="ercs")
                    nc.vector.reciprocal(rcs[:], sg_[:])
                    epi_rcs[v] = rcs
                return go

            def task_eext(v, pp, hl):
                def go():
                    acc = epi_acc[v]
                    rcs = epi_rcs[v]
                    hg = 4 * pp + hl
                    r0 = 64 * (hl // 2) + 32 * (hl % 2)
                    st = sb.tile([F, D], BF16, tag="st", bufs=4, name="st")
                    nc.vector.tensor_scalar_mul(
                        st[:], acc[r0:r0 + F, 65 * hl:65 * hl + D],
                        rcs[r0:r0 + F, hl:hl + 1])
                    pst = next_psc()[:].bitcast(BF16)
                    nc.tensor.transpose(pst[:D, 0:F], st[:], identh[:F, :F])
                    nc.scalar.copy(
                        ocfT[hg // 2][64 * (hg % 2):64 * (hg % 2) + D,
                                      F * v:F * (v + 1)],
                        pst[:D, 0:F])
                return go

            def epi_tasks(v):
                tasks = [task_qbd(v)]
                for pp in range(3):
                    for c in range(NKC):
                        tasks.append(task_epi_sc(v, pp, c))
                        if c >= 3:
                            tasks.append(task_epi_av(v, pp, c - 3))
                    for c in range(NKC - 3, NKC):
                        tasks.append(task_epi_av(v, pp, c))
                    tasks.append(task_esum(v, pp))
                    tasks += [task_eext(v, pp, hl) for hl in range(4)]
                return tasks

            # ---------------- finale: CLS adapter + cross-frame out-proj ----
            def emit_finale():
                ycls = sb.tile([S, E], F32, tag="ycls", bufs=1, name="ycls")
                nc.sync.dma_start(out=ycls[:], in_=y_d[:, 0, :])
                yclsh = sb.tile([S, E], BF16, tag="yclsh", bufs=1,
                                name="yclsh")
                nc.vector.tensor_copy(yclsh[:], ycls[:])
                yclsT = []
                for k in range(KT):
                    pst = next_psc()[:].bitcast(BF16)
                    nc.tensor.transpose(pst[:, :S], yclsh[:, 128 * k:128 * (k + 1)],
                                        identh[:S, :S])
                    t_ = sb.tile([128, S], BF16, tag="yclsT", bufs=6,
                                 name=f"yclsT{k}")
                    nc.scalar.copy(t_[:], pst[:, :S])
                    yclsT.append(t_)
                ps8 = psav.tile([128, 512], F32, tag="pav", name="ps8")
                for k in range(KT):
                    nc.tensor.matmul(ps8[:R, :S], dwtt[:, R * k:R * (k + 1)],
                                     yclsT[k][:],
                                     start=(k == 0), stop=(k == KT - 1))
                z = sb.tile([R, S], F32, tag="z8", name="z8")
                if has_down_bias:
                    nc.scalar.activation(z[:], ps8[:R, :S], AF.Identity,
                                         bias=bdown[:, 0:1])
                else:
                    nc.scalar.copy(z[:], ps8[:R, :S])
                en = sb.tile([R, S], F32, tag="sg8", name="sg8")
                nc.scalar.activation(en[:], z[:], AF.Exp, scale=-1.702)
                nc.vector.tensor_scalar_add(en[:], en[:], 1.0)
                rec = sb.tile([R, S], F32, tag="rec8", name="rec8")
                nc.vector.reciprocal_approx_fast(rec[:], en[:])
                gq = sb.tile([R, S], BF16, tag="gq8", name="gq8")
                nc.vector.tensor_tensor(out=gq[:], in0=z[:], in1=rec[:],
                                        op=MUL)
                cn = sb.tile([S, E], F32, tag="cn", bufs=1, name="cn")
                for m in range(KT):
                    ps = psmm.tile([128, 512], F32, tag="pmm", name="psf")
                    nc.tensor.matmul(ps[:, :S], uwt[:, 128 * m:128 * (m + 1)],
                                     gq[:], start=True, stop=False)
                    for k in range(KT):
                        nc.tensor.matmul(ps[:, :S],
                                         wot[:, E * k + 128 * m:
                                              E * k + 128 * (m + 1)],
                                         ocfT[k][:], start=False,
                                         stop=(k == KT - 1))
                    cnT = sb.tile([128, S], F32, tag="cnT", bufs=2, name="cnT")
                    if has_cls_bias:
                        nc.scalar.activation(cnT[:], ps[:, :S], AF.Identity,
                                             bias=bcls[:, m:m + 1])
                    else:
                        if m % 2 == 0:
                            nc.scalar.copy(cnT[:], ps[:, :S])
                        else:
                            nc.vector.tensor_copy(cnT[:], ps[:, :S])
                    pst = next_psc()
                    nc.tensor.transpose(pst[:S, 0:128], cnT[:], identf[:, :])
                    if m % 2 == 0:
                        nc.vector.tensor_copy(cn[:, 128 * m:128 * (m + 1)],
                                              pst[:S, 0:128])
                    else:
                        nc.scalar.copy(cn[:, 128 * m:128 * (m + 1)],
                                       pst[:S, 0:128])
                nc.sync.dma_start(out=y_d[:, 0, :], in_=cn[:])

            # ---------------- emission schedule ----------------
            load_xt(2)
            epi0 = None
            ntt = (GT + 127) // 128
            for p in range(NP):
                g = (2 * p) // G
                if (2 * p) % G == 0:
                    attnTg[g] = [sb.tile([128, GT], BF16, tag="gt", bufs=12,
                                         name=f"gt{g}_{k}")
                                 for k in range(KT)]
                if p + 3 < NP:
                    load_xt(p + 3)
                emit_pair_qkv(p)
                emit_pair_v(p)
                if p == 11:
                    # tail: interleave the last pair's attention, the last
                    # group's out-proj, and the second video's epilogue so
                    # exp- and DMA-bound chains hide behind tensor-dense work
                    at = attn_tasks(11)
                    op_ = [task_group_tile(2, tt) for tt in range(ntt)]
                    ep = epi_tasks(1)
                    ia = ib = 0
                    while ia < len(at) or ib < len(ep):
                        if ia < len(at):
                            pend.append(at[ia]); ia += 1
                        for _ in range(2):
                            if ib < len(ep):
                                pend.append(ep[ib]); ib += 1
                        if ia >= len(at) and op_:
                            pend.append(op_.pop(0))
                    pend.extend(op_)
                else:
                    pend.extend(attn_tasks(p))
                    if p % 4 == 3:
                        for tt in range(ntt):
                            pend.append(task_group_tile(g, tt))
                if p == 5:
                    epi0 = epi_tasks(0)
                if epi0:
                    pend.extend(epi0[:45])
                    epi0 = epi0[45:]
                # bound the backlog: stale tasks past ~1 pair break the ring
                # buffers' reuse-distance assumptions
                while len(pend) > 36:
                    drain(1)
            drain(len(pend))
            emit_finale()

    nc.finalize()
    return nc


def _preprocess(x, in_proj_weight, in_proj_bias, out_proj_weight,
                out_proj_bias, lora_a, lora_b, down_w, down_b, up_w, up_b):
    w_comb = in_proj_weight.astype(np.float64) + \
        lora_b.astype(np.float64) @ lora_a.astype(np.float64)
    w_comb = w_comb.astype(np.float32)
    b_v = in_proj_bias[2 * E:3 * E].astype(np.float32)
    bias_row = out_proj_bias.astype(np.float32) + \
        b_v @ out_proj_weight.T.astype(np.float32)
    b_cls = bias_row + up_b.astype(np.float32)
    def _pack(w):  # [E, C] -> [128, KT*C] with k-major columns
        kt = w.reshape(KT, 128, -1)
        return _bf(kt.transpose(1, 0, 2).reshape(128, -1))

    p = {
        "w_qkt": _bf(np.ascontiguousarray(
            w_comb[0:2 * E].T.reshape(KT, 128, 2 * KT, 128)
            .transpose(2, 1, 0, 3).reshape(2 * KT, 128, KT * 128))),
        "w_vt": _pack(np.ascontiguousarray(w_comb[2 * E:3 * E].T)),
        "w_ot": _pack(np.ascontiguousarray(out_proj_weight.T)),
        "b_qk_t": _f32(in_proj_bias[0:2 * E].reshape(2 * KT, 128).T),
        "bias_row_o": _bf(bias_row.reshape(1, E)),
        "b_cls_t": _f32(b_cls.reshape(KT, 128).T),
        "down_wt": _pack(np.ascontiguousarray(down_w.T)),
        "b_down": _f32(down_b.reshape(R, 1)),
        "up_wt": _bf(up_w.T),
    }
    flags = (
        bool(np.any(in_proj_bias[0:2 * E])),
        bool(np.any(bias_row)),
        bool(np.any(down_b)),
        bool(np.any(b_cls)),
    )
    # xt per core: [NP, 128, KT*T2] with cols k-major, two seqs side by side
    xts = []
    xb = x.astype(ml_dtypes.bfloat16)
    for c in range(NCORES):
        xc = xb[S * c:S * (c + 1)]            # [S, T, E]
        xt = np.empty((NP, 128, KT * T2), dtype=ml_dtypes.bfloat16)
        for pi in range(NP):
            a = xc[2 * pi].T.reshape(KT, 128, T)      # [KT, 128, T]
            b = xc[2 * pi + 1].T.reshape(KT, 128, T)
            blk = np.concatenate([a, b], axis=2)      # [KT, 128, T2]
            xt[pi] = blk.transpose(1, 0, 2).reshape(128, KT * T2)
        xts.append(np.ascontiguousarray(xt))
    return p, flags, xts


def kernel(x, in_proj_weight, in_proj_bias, out_proj_weight, out_proj_bias,
           lora_a, lora_b, down_w, down_b, up_w, up_b,
           b, n_f, token_len, d_v):
    global _last_results
    x = np.asarray(x, dtype=np.float32)
    assert x.shape == (B * F, T, E), x.shape
    params, flags, xts = _preprocess(
        x, np.asarray(in_proj_weight), np.asarray(in_proj_bias),
        np.asarray(out_proj_weight), np.asarray(out_proj_bias),
        np.asarray(lora_a), np.asarray(lora_b),
        np.asarray(down_w), np.asarray(down_b),
        np.asarray(up_w), np.asarray(up_b))

    nc = _build(*flags)

    in_maps = []
    for c in range(NCORES):
        m = dict(params)
        m["xt"] = xts[c]
        in_maps.append(m)

    res = run_bass_kernel_spmd(nc, in_maps, list(range(NCORES)))
    _last_results = res
    out = np.concatenate([res.results[c]["y"] for c in range(NCORES)], axis=0)
    return out.astype(np.float32)


# revision 50
# speedup vs baseline: 1.1950x; 1.1950x over previous
"""Trainium2 Bass kernel for the LoRA-QKV + per-frame local attention +
cross-frame CLS attention + adapter module (nn_Attention sparse_attention).

Contract: kernel(**inputs) takes FULL unsharded inputs (as in
reference.setup_inputs()), shards the video batch over 8 NeuronCores
(2 videos = 24 frames per core), runs one SPMD Bass program, and returns
the FULL [192, 197, 768] fp32 output.

Math notes (exact algebra, not approximations):
  - qkv = x@(W + lora_b@lora_a).T + in_proj_bias  (LoRA folded on host)
  - v bias is folded through the out projection: attn@(v + 1 b_v^T) @ Wo^T
    = attn@v @ Wo^T + 1 (b_v @ Wo^T)^T, merged with out_proj_bias into one
    rank-1 bias row added via a K=1 matmul.
  - softmax computed without max subtraction (scores here are O(1); exp is
    well inside fp32 range), matching softmax exactly in exact arithmetic.
  - cross-frame attention outputs are normalized after the AV matmul
    (linearity of AV in the attention weights).
Matmuls run in bf16 with fp32 PSUM accumulation.

Performance structure (v3, 612.7us vs 1063us baseline on TRN2):
  - x is pre-transposed and pre-cast to bf16 on the host, fed as per-pair
    tiles xT [128, 6*394] (two seqs side by side); QKV weights are packed
    m-block-major so the first matmul only waits on a 192KB DMA.
  - QKV runs 2 seqs per matmul (N=394) with the weight tile stationary;
    merged exp covers both key chunks of a head in one activation (the
    PSUM zero-region rows are defined via persistent ring tiles).
  - emission is a software-pipelined task queue: per-head attention work of
    pair p-1 (scores -> exp -> AV -> normalize) is interleaved between the
    QKV/V-proj chunks of pair p, with every dependent tensor op placed >=2
    tasks behind its scalar/vector producer so the in-order tensor queue
    never head-blocks (keeps the PE near its top p-state).
  - k^T tiles stay SBUF-resident for the whole video; the cross-frame
    epilogue computes scores transposed ([keys, queries]) so the attention
    probabilities feed the AV matmul directly -- no PE transposes or
    PSUM->SBUF repacks -- with softmax sums from ones-columns in the spilled
    v tiles and normalization applied at extraction.
  - softmax reciprocal rows are broadcast to 64 partitions with the gpsimd
    partition_broadcast DMA (input must be at partition 0 on hardware).
  - v spills / reloads are gpsimd-issued DMAs (25ns dispatch vs ~700ns on
    sync), contiguous in a [keys, 780] layout.
"""

import sys
from collections import deque

sys.path.insert(0, "/opt/trn_rl_repo")

import numpy as np
import ml_dtypes

import concourse.bass as bass
import concourse.mybir as mybir
import concourse.tile as tile
from concourse import bacc
from concourse.bass_utils import run_bass_kernel_spmd
from concourse.masks import make_identity

F32 = mybir.dt.float32
BF16 = mybir.dt.bfloat16
AF = mybir.ActivationFunctionType
MUL = mybir.AluOpType.mult

NCORES = 8
B, F, T, E, H, D, R = 16, 12, 197, 768, 12, 64, 8
NV = B // NCORES          # videos per core = 2
S = NV * F                # seqs per core = 24
NP = S // 2               # seq pairs per core = 12
KT = E // 128             # 6 feature k-tiles
T2 = 2 * T                # 394: two seqs of tokens side by side
G = 8                     # seqs per out-proj group
NG = S // G
GT = G * T                # tokens per group = 1576
SCALE = float(D) ** -0.5
TQ = T + 1                # 198: pair column stride inside ps_o (4B-aligned)
FT = F * T                # keys per video for cross-frame = 2364
NKC = 2 * F               # irregular key chunks per video (128/69 per frame)

_last_results = None  # test harness reads exec_time_ns from here


def _bf(x):
    return np.ascontiguousarray(x.astype(ml_dtypes.bfloat16))


def _f32(x):
    return np.ascontiguousarray(x.astype(np.float32))


def _build(has_qk_bias, has_orow_bias, has_down_bias, has_cls_bias):
    nc = bacc.Bacc("TRN2", target_bir_lowering=False, debug=False,
                   num_devices=NCORES)

    xt_d = nc.declare_dram_parameter("xt", [NP, 128, KT * T2], BF16,
                                     isOutput=False)
    wqk_d = nc.declare_dram_parameter("w_qkt", [2 * KT, 128, KT * 128], BF16,
                                      isOutput=False)
    wv_d = nc.declare_dram_parameter("w_vt", [128, KT * E], BF16,
                                     isOutput=False)
    wo_d = nc.declare_dram_parameter("w_ot", [128, KT * E], BF16,
                                     isOutput=False)
    bqk_d = nc.declare_dram_parameter("b_qk_t", [128, 2 * KT], F32,
                                      isOutput=False)
    brow_d = nc.declare_dram_parameter("bias_row_o", [1, E], BF16,
                                       isOutput=False)
    bcls_d = nc.declare_dram_parameter("b_cls_t", [128, KT], F32,
                                       isOutput=False)
    dwt_d = nc.declare_dram_parameter("down_wt", [128, KT * R], BF16,
                                      isOutput=False)
    bdown_d = nc.declare_dram_parameter("b_down", [R, 1], F32, isOutput=False)
    uwt_d = nc.declare_dram_parameter("up_wt", [R, E], BF16, isOutput=False)

    y_d = nc.declare_dram_parameter("y", [S, T, E], F32, isOutput=True)
    vs_d = nc.dram_tensor("v_scr", [NV, FT, H * (D + 1)], BF16)

    y_flat = y_d.ap().rearrange("a b c -> (a b) c")

    with tile.TileContext(nc) as tc:
        with (
            tc.tile_pool(name="cst", bufs=1) as cst,
            tc.tile_pool(name="sb", bufs=2) as sb,
            tc.tile_pool(name="psmm", bufs=2, space="PSUM") as psmm,
            tc.tile_pool(name="pssc", bufs=1, space="PSUM") as pssc,
            tc.tile_pool(name="psav", bufs=3, space="PSUM") as psav,
            tc.tile_pool(name="psep", bufs=1, space="PSUM") as psep,
        ):
            # ---------------- input prefetch + constants ----------------
            # first two xt pair-tiles go out before the weights so the QKV
            # pipeline can start as soon as wqk lands
            xt_tiles = {}

            def load_xt(p):
                t_ = sb.tile([128, KT * T2], BF16, tag="xt", bufs=4,
                             name=f"xt{p}")
                nc.sync.dma_start(out=t_[:], in_=xt_d[p])
                xt_tiles[p] = t_

            load_xt(0)
            wqkm = []
            qs = [nc.sync, nc.scalar, nc.gpsimd]
            for m in range(2 * KT):
                t_ = cst.tile([128, KT * 128], BF16, tag=f"wqkm{m}",
                              name=f"wqkm{m}")
                qs[m % 3].dma_start(out=t_[:], in_=wqk_d[m])
                wqkm.append(t_)
            wvt = cst.tile([128, KT * E], BF16, tag="wvt")
            nc.gpsimd.dma_start(out=wvt[:], in_=wv_d[:, :])
            load_xt(1)
            wot = cst.tile([128, KT * E], BF16, tag="wot")
            nc.scalar.dma_start(out=wot[:], in_=wo_d[:, :])
            bqk = cst.tile([128, 2 * KT], F32, tag="bqk")
            nc.sync.dma_start(out=bqk[:], in_=bqk_d[:, :])
            brow = cst.tile([1, E], BF16, tag="brow")
            nc.sync.dma_start(out=brow[:], in_=brow_d[:, :])
            bcls = cst.tile([128, KT], F32, tag="bcls")
            nc.sync.dma_start(out=bcls[:], in_=bcls_d[:, :])
            dwtt = cst.tile([128, KT * R], BF16, tag="dwtt")
            nc.sync.dma_start(out=dwtt[:], in_=dwt_d[:, :])
            bdown = cst.tile([R, 1], F32, tag="bdown")
            nc.sync.dma_start(out=bdown[:], in_=bdown_d[:, :])
            uwt = cst.tile([R, E], BF16, tag="uwt")
            nc.sync.dma_start(out=uwt[:], in_=uwt_d[:, :])

            identh = cst.tile([128, 128], BF16, tag="identh")
            make_identity(nc, identh[:])
            identf = cst.tile([128, 128], F32, tag="identf")
            make_identity(nc, identf[:])
            ones_h = cst.tile([97, 128], BF16, tag="ones_h")
            nc.vector.memset(ones_h[:], 1.0)

            # persistent cross-frame state
            q1s = [cst.tile([128, S], BF16, tag=f"q1s{m}", name=f"q1s{m}")
                   for m in range(KT)]
            ocfT = [cst.tile([128, S], BF16, tag=f"ocfT{k}", name=f"ocfT{k}")
                    for k in range(KT)]
            qbd = [[None] * KT for _ in range(NV)]

            # Persistent ring buffers: tiles whose pad regions must stay
            # defined across reuses. Using one tile id per slot keeps the
            # race detector's happens-before tracking intact (subtile deps)
            # while the one-time initialization of the pad bytes persists.
            vsl_ring = []
            for i in range(6):
                t_ = cst.tile([128, H * (D + 1)], BF16, tag=f"vsl{i}",
                              name=f"vsl{i}")
                t3 = t_[:].rearrange("p (h d) -> p h d", h=H)
                nc.gpsimd.memset(t3[:, :, D:D + 1], 1.0)
                vsl_ring.append(t_)
            # two persistent PSUM score banks: the merged exp's zero-region
            # rows read bytes only this one-time memset wrote (exp(0)=1,
            # never consumed downstream)
            psc_ring = []
            for i in range(2):
                t_ = pssc.tile([128, 512], F32, tag=f"psc{i}", name=f"psc{i}")
                nc.vector.memset(t_[:], 0.0)
                psc_ring.append(t_)
            cnt = {"psc": 0, "vsl": 0, "smt": 0}

            def next_psc():
                t_ = psc_ring[cnt["psc"] % 2]
                cnt["psc"] += 1
                return t_

            # ---------------- task machinery ----------------
            pend = deque()

            def drain(n):
                for _ in range(n):
                    if pend:
                        pend.popleft()()

            qtiles = {}     # (p, m 0..5) -> q^T tile [128, T2]
            ktiles = {}     # (p, j 0..5) -> k^T tile [128, T2]
            vslabs = {}     # (s, ci) -> vslab tile
            pT_store = {}
            smt_cur = {}
            pair_ctx = {}
            attnTg = {}
            epi_acc = {}
            epi_rcs = {}
            epi_vt = {}
            epi_pl = {}

            # ---------------- per-pair inline emission ----------------
            def emit_pair_qkv(p):
                xt = xt_tiles[p]
                for m in range(2 * KT):
                    ps = psmm.tile([128, 512], F32, tag="pmm", name="psqkv")
                    for k in range(KT):
                        nc.tensor.matmul(ps[:, :T2],
                                         wqkm[m][:, 128 * k:128 * (k + 1)],
                                         xt[:, T2 * k:T2 * (k + 1)],
                                         start=(k == 0), stop=(k == KT - 1))
                    if m < KT:
                        t_ = sb.tile([128, T2], BF16, tag="qt", bufs=12,
                                     name=f"qt{p}_{m}")
                    else:
                        t_ = sb.tile([128, T2], BF16, tag="kt", bufs=54,
                                     name=f"kt{p}_{m - KT}")
                    on_scalar = (m % 3 == 0)
                    if has_qk_bias:
                        if on_scalar:
                            nc.scalar.activation(t_[:], ps[:, :T2], AF.Identity,
                                                 bias=bqk[:, m:m + 1])
                        else:
                            nc.vector.tensor_scalar_add(t_[:], ps[:, :T2],
                                                        bqk[:, m:m + 1])
                    else:
                        if on_scalar:
                            nc.scalar.copy(t_[:], ps[:, :T2])
                        else:
                            nc.vector.tensor_copy(t_[:], ps[:, :T2])
                    if m < KT:
                        qtiles[(p, m)] = t_
                        # CLS queries: col 0 of each seq half
                        src = t_[:].rearrange("p (b c) -> p b c", c=T)[:, :, 0:1]
                        nc.gpsimd.tensor_copy(
                            q1s[m][:, 2 * p:2 * p + 2],
                            src.rearrange("p b c -> p (b c)"))
                    else:
                        ktiles[(p, m - KT)] = t_
                    drain(4)

            def emit_pair_v(p):
                xt = xt_tiles[p]
                for s in (2 * p, 2 * p + 1):
                    v, f = s // F, s % F
                    for ci in range(2):
                        off = 197 * (s % 2) + 128 * ci
                        rows = 128 if ci == 0 else 69
                        psA = psmm.tile([128, 512], F32, tag="pmm", name="psva")
                        psB = psmm.tile([128, 512], F32, tag="pmm", name="psvb")
                        for k in range(KT):
                            lh = xt[:, T2 * k + off:T2 * k + off + rows]
                            nc.tensor.matmul(psA[:rows, :512], lh,
                                             wvt[:, E * k:E * k + 512],
                                             start=(k == 0), stop=(k == KT - 1))
                            nc.tensor.matmul(psB[:rows, :256], lh,
                                             wvt[:, E * k + 512:E * (k + 1)],
                                             start=(k == 0), stop=(k == KT - 1))
                        vt = vsl_ring[cnt["vsl"] % 6]
                        cnt["vsl"] += 1
                        vt3 = vt[:].rearrange("p (h d) -> p h d", h=H)
                        # ones columns persist from the pre-init pass
                        srcA = psA[:rows, :512].rearrange("p (h d) -> p h d",
                                                          h=8)
                        srcB = psB[:rows, :256].rearrange("p (h d) -> p h d",
                                                          h=4)
                        if ci == 0:
                            nc.vector.tensor_copy(vt3[:rows, 0:8, 0:D], srcA)
                            nc.scalar.copy(vt3[:rows, 8:12, 0:D], srcB)
                        else:
                            nc.scalar.copy(vt3[:rows, 0:8, 0:D], srcA)
                            nc.vector.tensor_copy(vt3[:rows, 8:12, 0:D], srcB)
                        nc.gpsimd.dma_start(
                            out=vs_d[v, T * f + 128 * ci:T * f + 128 * ci + rows, :],
                            in_=vt[:rows, :])
                        vslabs[(s, ci)] = vt
                        drain(4)

            # ---------------- local attention tasks ----------------
            def task_sc(s, j, i):
                def go():
                    p, sc = s // 2, 197 * (s % 2)
                    kt_, qt_ = ktiles[(p, j)], qtiles[(p, j)]
                    r0 = 64 * i
                    ps_s = next_psc()
                    nc.tensor.matmul(ps_s[:, 0:T], kt_[r0:r0 + 64, sc:sc + 128],
                                     qt_[r0:r0 + 64, sc:sc + T],
                                     start=True, stop=True)
                    nc.tensor.matmul(ps_s[0:69, T:T2],
                                     kt_[r0:r0 + 64, sc + 128:sc + T],
                                     qt_[r0:r0 + 64, sc:sc + T],
                                     start=True, stop=True)
                    pT = sb.tile([128, T2], BF16, tag="pT", bufs=8,
                                 name=f"pT{s}_{j}_{i}")
                    # rows 69:128 of cols T:T2 are the bank's zero region:
                    # exp(0)=1, never read by the K=69 AV matmul
                    nc.scalar.activation(pT[:], ps_s[:, 0:T2], AF.Exp,
                                         scale=SCALE)
                    pT_store[(s, j, i)] = pT
                return go

            def task_av(s, j):
                def go():
                    g, sg = s // G, s % G
                    ps_o = psav.tile([128, 512], F32, tag="pav", name="ps_o")
                    for i in range(2):
                        h = 2 * j + i
                        pT = pT_store.pop((s, j, i))
                        vs0, vs1 = vslabs[(s, 0)], vslabs[(s, 1)]
                        nc.tensor.matmul(ps_o[:D + 1, TQ * i:TQ * i + T],
                                         vs0[:, (D + 1) * h:(D + 1) * (h + 1)],
                                         pT[:, 0:T], start=True, stop=False)
                        nc.tensor.matmul(ps_o[:D + 1, TQ * i:TQ * i + T],
                                         vs1[0:69, (D + 1) * h:(D + 1) * (h + 1)],
                                         pT[0:69, T:T2], start=False, stop=True)
                    if j % 2 == 0:
                        smt_cur[s] = sb.tile([1, 2 * T2], F32, tag="smt",
                                             bufs=3, name="smt")
                        pair_ctx[s] = []
                    smt = smt_cur[s]
                    base = T2 * (j % 2)
                    src = ps_o[D:D + 1, 0:2 * TQ].rearrange(
                        "p (b c) -> p b c", c=TQ)[:, :, 0:T]
                    dst = smt[0:1, base:base + T2].rearrange(
                        "p (b c) -> p b c", c=T)
                    if j % 2 == 0:
                        nc.scalar.copy(dst, src)
                    else:
                        nc.vector.tensor_copy(dst, src)
                    pair_ctx[s].append((j, base, ps_o))
                return go

            def task_fin(s):
                def go():
                    g, sg = s // G, s % G
                    gt = attnTg[g]
                    smt = smt_cur[s]
                    entries = pair_ctx[s]
                    rinv = sb.tile([1, 2 * T2], F32, tag="rinv", bufs=2,
                                   name="rinv")
                    nc.vector.reciprocal_approx_fast(rinv[:], smt[:])
                    for (jj, base, ps_o) in entries:
                        rb = sb.tile([D, T2], F32, tag="rb", bufs=3,
                                     name="rb")
                        nc.gpsimd.partition_broadcast(
                            rb[:], rinv[0:1, base:base + T2])
                        for i in range(2):
                            nc.vector.tensor_tensor(
                                out=gt[jj][64 * i:64 * i + 64,
                                           T * sg:T * (sg + 1)],
                                in0=ps_o[0:D, TQ * i:TQ * i + T],
                                in1=rb[:, T * i:T * (i + 1)],
                                op=MUL)
                    pair_ctx[s] = []
                return go

            def attn_tasks(p):
                out = []
                for s in (2 * p, 2 * p + 1):
                    for j in range(H // 2):
                        out.append(task_sc(s, j, 0))
                        out.append(task_sc(s, j, 1))
                        if j >= 1:
                            out.append(task_av(s, j - 1))
                            if j % 2 == 0:
                                out.append(task_fin(s))
                    out.append(task_av(s, H // 2 - 1))
                    out.append(task_fin(s))
                return out

            # ---------------- out-proj tasks ----------------
            def task_group_tile(g, tt):
                def go():
                    gt = attnTg[g]
                    c0 = 128 * tt
                    rows = min(128, GT - c0)
                    psA = psmm.tile([128, 512], F32, tag="pmm", name="psoa")
                    psB = psmm.tile([128, 512], F32, tag="pmm", name="psob")
                    laststop = not has_orow_bias
                    for k in range(KT):
                        lh = gt[k][:, c0:c0 + rows]
                        nc.tensor.matmul(psA[:rows, :512], lh,
                                         wot[:, E * k:E * k + 512],
                                         start=(k == 0),
                                         stop=(k == KT - 1 and laststop))
                        nc.tensor.matmul(psB[:rows, :256], lh,
                                         wot[:, E * k + 512:E * (k + 1)],
                                         start=(k == 0),
                                         stop=(k == KT - 1 and laststop))
                    if has_orow_bias:
                        nc.tensor.matmul(psA[:rows, :512], ones_h[:, :rows],
                                         brow[:, 0:512], start=False, stop=True)
                        nc.tensor.matmul(psB[:rows, :256], ones_h[:, :rows],
                                         brow[:, 512:768], start=False,
                                         stop=True)
                    of = sb.tile([128, E], F32, tag="of", bufs=2, name="of")
                    nc.scalar.copy(of[:rows, 0:512], psA[:rows, :512])
                    nc.vector.tensor_copy(of[:rows, 512:768], psB[:rows, :256])
                    r0 = GT * g + c0
                    nc.sync.dma_start(out=y_flat[r0:r0 + rows, :],
                                      in_=of[:rows, :])
                return go

            # ---------------- cross-frame epilogue tasks ----------------
            # Transposed formulation: scores land as [keys, queries] so the
            # attention probabilities feed the AV matmul directly as the
            # stationary operand (no PE transposes / PSUM->SBUF repacks).
            # Three passes of 4 heads (queries at 32-aligned row blocks);
            # softmax sums come from ones-columns in the v tiles and the
            # normalization is applied at extraction (AV is linear in the
            # attention weights).
            def task_qbd(v):
                def go():
                    for m in range(KT):
                        bd = cst.tile([128, 64], BF16, tag=f"qbd{v}_{m}",
                                      name=f"qbd{v}_{m}")
                        nc.gpsimd.memset(bd[:], 0.0)
                        nc.gpsimd.tensor_copy(bd[0:64, 0:F],
                                              q1s[m][0:64, F * v:F * (v + 1)])
                        nc.gpsimd.tensor_copy(bd[64:128, 32:32 + F],
                                              q1s[m][64:128, F * v:F * (v + 1)])
                        qbd[v][m] = bd
                    _issue_vt(v, 0, 0)
                    _issue_vt(v, 0, 1)
                return go

            def _chunk_geo(c):
                f2, ci = c // 2, c % 2
                return (T * f2 + 128 * ci, 128 if ci == 0 else 69, f2, ci)

            def _issue_vt(v, pp, c):
                c0, rows, f2, ci = _chunk_geo(c)
                vt_ = sb.tile([128, 4 * (D + 1)], BF16, tag="vte", bufs=8,
                              name="vte")
                nc.gpsimd.dma_start(out=vt_[:rows, :],
                                    in_=vs_d[v, c0:c0 + rows,
                                             4 * (D + 1) * pp:
                                             4 * (D + 1) * (pp + 1)])
                epi_vt[(v, pp, c)] = vt_

            def task_epi_sc(v, pp, c):
                def go():
                    c0, rows, f2, ci = _chunk_geo(c)
                    if c + 2 < NKC:
                        _issue_vt(v, pp, c + 2)
                    elif pp < 2:
                        _issue_vt(v, pp + 1, c + 2 - NKC)
                    if c == 0:
                        epi_acc[v] = psep.tile([128, 4 * (D + 1)], F32,
                                               tag="peav", name="peav")
                    ps = next_psc()
                    kt_off = 197 * (f2 % 2) + 128 * ci
                    for pl in range(2):
                        jp = 2 * pp + pl
                        kt_ = ktiles[(6 * v + f2 // 2, jp)]
                        nc.tensor.matmul(
                            ps[:rows, 64 * pl:64 * pl + 64],
                            kt_[:, kt_off:kt_off + rows], qbd[v][jp][:],
                            start=True, stop=True)
                    pl_t = sb.tile([128, 128], BF16, tag="plT", bufs=8,
                                   name="plT")
                    nc.scalar.activation(pl_t[:rows, :], ps[:rows, 0:128],
                                         AF.Exp, scale=SCALE)
                    epi_pl[(v, pp, c)] = pl_t
                return go

            def task_epi_av(v, pp, c):
                def go():
                    c0, rows, f2, ci = _chunk_geo(c)
                    pl_t = epi_pl.pop((v, pp, c))
                    vt_ = epi_vt.pop((v, pp, c))
                    nc.tensor.matmul(epi_acc[v][0:128, 0:4 * (D + 1)],
                                     pl_t[:rows, :], vt_[:rows, :],
                                     start=(c == 0), stop=(c == NKC - 1))
                return go

            def task_esum(v, pp):
                def go():
                    acc = epi_acc[v]
                    sg_ = sb.tile([128, 4], F32, tag="esum", bufs=2,
                                  name="esum")
                    nc.vector.tensor_copy(
                        sg_[:], acc[:, :].rearrange("p (h d) -> p h d",
                                                    h=4)[:, :, D:D + 1]
                        .rearrange("p h d -> p (h d)"))
                    rcs = sb.tile([128, 4], F32, tag="ercs", bufs=2,
                                  name="ercs")
                    nc.vector.reciprocal(rcs[:], sg_[:])
                    epi_rcs[v] = rcs
                return go

            def task_eext(v, pp, hl):
                def go():
                    acc = epi_acc[v]
                    rcs = epi_rcs[v]
                    hg = 4 * pp + hl
                    r0 = 64 * (hl // 2) + 32 * (hl % 2)
                    st = sb.tile([F, D], BF16, tag="st", bufs=4, name="st")
                    nc.vector.tensor_scalar_mul(
                        st[:], acc[r0:r0 + F, 65 * hl:65 * hl + D],
                        rcs[r0:r0 + F, hl:hl + 1])
                    pst = next_psc()[:].bitcast(BF16)
                    nc.tensor.transpose(pst[:D, 0:F], st[:], identh[:F, :F])
                    nc.scalar.copy(
                        ocfT[hg // 2][64 * (hg % 2):64 * (hg % 2) + D,
                                      F * v:F * (v + 1)],
                        pst[:D, 0:F])
                return go

            def epi_tasks(v):
                tasks = [task_qbd(v)]
                for pp in range(3):
                    for c in range(NKC):
                        tasks.append(task_epi_sc(v, pp, c))
                        if c >= 3:
                            tasks.append(task_epi_av(v, pp, c - 3))
                    for c in range(NKC - 3, NKC):
                        tasks.append(task_epi_av(v, pp, c))
                    tasks.append(task_esum(v, pp))
                    tasks += [task_eext(v, pp, hl) for hl in range(4)]
                return tasks

            # ---------------- finale: CLS adapter + cross-frame out-proj ----
            def emit_finale():
                ycls = sb.tile([S, E], F32, tag="ycls", bufs=1, name="ycls")
                nc.sync.dma_start(out=ycls[:], in_=y_d[:, 0, :])
                yclsh = sb.tile([S, E], BF16, tag="yclsh", bufs=1,
                                name="yclsh")
                nc.vector.tensor_copy(yclsh[:], ycls[:])
                yclsT = []
                for k in range(KT):
                    pst = next_psc()[:].bitcast(BF16)
                    nc.tensor.transpose(pst[:, :S], yclsh[:, 128 * k:128 * (k + 1)],
                                        identh[:S, :S])
                    t_ = sb.tile([128, S], BF16, tag="yclsT", bufs=6,
                                 name=f"yclsT{k}")
                    nc.scalar.copy(t_[:], pst[:, :S])
                    yclsT.append(t_)
                ps8 = psav.tile([128, 512], F32, tag="pav", name="ps8")
                for k in range(KT):
                    nc.tensor.matmul(ps8[:R, :S], dwtt[:, R * k:R * (k + 1)],
                                     yclsT[k][:],
                                     start=(k == 0), stop=(k == KT - 1))
                z = sb.tile([R, S], F32, tag="z8", name="z8")
                if has_down_bias:
                    nc.scalar.activation(z[:], ps8[:R, :S], AF.Identity,
                                         bias=bdown[:, 0:1])
                else:
                    nc.scalar.copy(z[:], ps8[:R, :S])
                en = sb.tile([R, S], F32, tag="sg8", name="sg8")
                nc.scalar.activation(en[:], z[:], AF.Exp, scale=-1.702)
                nc.vector.tensor_scalar_add(en[:], en[:], 1.0)
                rec = sb.tile([R, S], F32, tag="rec8", name="rec8")
                nc.vector.reciprocal_approx_fast(rec[:], en[:])
                gq = sb.tile([R, S], BF16, tag="gq8", name="gq8")
                nc.vector.tensor_tensor(out=gq[:], in0=z[:], in1=rec[:],
                                        op=MUL)
                cn = sb.tile([S, E], F32, tag="cn", bufs=1, name="cn")
                for m in range(KT):
                    ps = psmm.tile([128, 512], F32, tag="pmm", name="psf")
                    nc.tensor.matmul(ps[:, :S], uwt[:, 128 * m:128 * (m + 1)],
                                     gq[:], start=True, stop=False)
                    for k in range(KT):
                        nc.tensor.matmul(ps[:, :S],
                                         wot[:, E * k + 128 * m:
                                              E * k + 128 * (m + 1)],
                                         ocfT[k][:], start=False,
                                         stop=(k == KT - 1))
                    cnT = sb.tile([128, S], F32, tag="cnT", bufs=2, name="cnT")
                    if has_cls_bias:
                        nc.scalar.activation(cnT[:], ps[:, :S], AF.Identity,
                                             bias=bcls[:, m:m + 1])
                    else:
                        if m % 2 == 0:
                            nc.scalar.copy(cnT[:], ps[:, :S])
                        else:
                            nc.vector.tensor_copy(cnT[:], ps[:, :S])
                    pst = next_psc()
                    nc.tensor.transpose(pst[:S, 0:128], cnT[:], identf[:, :])
                    if m % 2 == 0:
                        nc.vector.tensor_copy(cn[:, 128 * m:128 * (m + 1)],
                                              pst[:S, 0:128])
                    else:
                        nc.scalar.copy(cn[:, 128 * m:128 * (m + 1)],
                                       pst[:S, 0:128])
                nc.sync.dma_start(out=y_d[:, 0, :], in_=cn[:])

            # ---------------- emission schedule ----------------
            load_xt(2)
            epi0 = None
            ntt = (GT + 127) // 128
            for p in range(NP):
                g = (2 * p) // G
                if (2 * p) % G == 0:
                    attnTg[g] = [sb.tile([128, GT], BF16, tag="gt", bufs=12,
                                         name=f"gt{g}_{k}")
                                 for k in range(KT)]
                if p + 3 < NP:
                    load_xt(p + 3)
                emit_pair_qkv(p)
                emit_pair_v(p)
                if p == 11:
                    # tail: interleave the last pair's attention, the last
                    # group's out-proj, and the second video's epilogue so
                    # exp- and DMA-bound chains hide behind tensor-dense work
                    at = attn_tasks(11)
                    op_ = [task_group_tile(2, tt) for tt in range(ntt)]
                    ep = epi_tasks(1)
                    ia = ib = 0
                    while ia < len(at) or ib < len(ep):
                        if ia < len(at):
                            pend.append(at[ia]); ia += 1
                        for _ in range(2):
                            if ib < len(ep):
                                pend.append(ep[ib]); ib += 1
                        if ia >= len(at) and op_:
                            pend.append(op_.pop(0))
                    pend.extend(op_)
                else:
                    pend.extend(attn_tasks(p))
                    if p % 4 == 3:
                        for tt in range(ntt):
                            pend.append(task_group_tile(g, tt))
                if p == 5:
                    epi0 = epi_tasks(0)
                if epi0:
                    pend.extend(epi0[:45])
                    epi0 = epi0[45:]
                # bound the backlog: stale tasks past ~1 pair break the ring
                # buffers' reuse-distance assumptions
                while len(pend) > 36:
                    drain(1)
            drain(len(pend))
            emit_finale()

    nc.finalize()
    return nc


def _preprocess(x, in_proj_weight, in_proj_bias, out_proj_weight,
                out_proj_bias, lora_a, lora_b, down_w, down_b, up_w, up_b):
    w_comb = in_proj_weight.astype(np.float64) + \
        lora_b.astype(np.float64) @ lora_a.astype(np.float64)
    w_comb = w_comb.astype(np.float32)
    b_v = in_proj_bias[2 * E:3 * E].astype(np.float32)
    bias_row = out_proj_bias.astype(np.float32) + \
        b_v @ out_proj_weight.T.astype(np.float32)
    b_cls = bias_row + up_b.astype(np.float32)
    def _pack(w):  # [E, C] -> [128, KT*C] with k-major columns
        kt = w.reshape(KT, 128, -1)
        return _bf(kt.transpose(1, 0, 2).reshape(128, -1))

    p = {
        "w_qkt": _bf(np.ascontiguousarray(
            w_comb[0:2 * E].T.reshape(KT, 128, 2 * KT, 128)
            .transpose(2, 1, 0, 3).reshape(2 * KT, 128, KT * 128))),
        "w_vt": _pack(np.ascontiguousarray(w_comb[2 * E:3 * E].T)),
        "w_ot": _pack(np.ascontiguousarray(out_proj_weight.T)),
        "b_qk_t": _f32(in_proj_bias[0:2 * E].reshape(2 * KT, 128).T),
        "bias_row_o": _bf(bias_row.reshape(1, E)),
        "b_cls_t": _f32(b_cls.reshape(KT, 128).T),
        "down_wt": _pack(np.ascontiguousarray(down_w.T)),
        "b_down": _f32(down_b.reshape(R, 1)),
        "up_wt": _bf(up_w.T),
    }
    flags = (
        bool(np.any(in_proj_bias[0:2 * E])),
        bool(np.any(bias_row)),
        bool(np.any(down_b)),
        bool(np.any(b_cls)),
    )
    # xt per core: [NP, 128, KT*T2] with cols k-major, two seqs side by side
    xts = []
    xb = x.astype(ml_dtypes.bfloat16)
    for c in range(NCORES):
        xc = xb[S * c:S * (c + 1)]            # [S, T, E]
        xt = np.empty((NP, 128, KT * T2), dtype=ml_dtypes.bfloat16)
        for pi in range(NP):
            a = xc[2 * pi].T.reshape(KT, 128, T)      # [KT, 128, T]
            b = xc[2 * pi + 1].T.reshape(KT, 128, T)
            blk = np.concatenate([a, b], axis=2)      # [KT, 128, T2]
            xt[pi] = blk.transpose(1, 0, 2).reshape(128, KT * T2)
        xts.append(np.ascontiguousarray(xt))
    return p, flags, xts


def kernel(x, in_proj_weight, in_proj_bias, out_proj_weight, out_proj_bias,
           lora_a, lora_b, down_w, down_b, up_w, up_b,
           b, n_f, token_len, d_v):
    global _last_results
    x = np.asarray(x, dtype=np.float32)
    assert x.shape == (B * F, T, E), x.shape
    params, flags, xts = _preprocess(
        x, np.asarray(in_proj_weight), np.asarray(in_proj_bias),
        np.asarray(out_proj_weight), np.asarray(out_proj_bias),
        np.asarray(lora_a), np.asarray(lora_b),
        np.asarray(down_w), np.asarray(down_b),
        np.asarray(up_w), np.asarray(up_b))

    nc = _build(*flags)

    in_maps = []
    for c in range(NCORES):
        m = dict(params)
        m["xt"] = xts[c]
        in_maps.append(m)

    res = run_bass_kernel_spmd(nc, in_maps, list(range(NCORES)))
    _last_results = res
    out = np.concatenate([res.results[c]["y"] for c in range(NCORES)], axis=0)
    return out.astype(np.float32)


# revision 52
# speedup vs baseline: 1.2073x; 1.0103x over previous
"""Trainium2 Bass kernel for the LoRA-QKV + per-frame local attention +
cross-frame CLS attention + adapter module (nn_Attention sparse_attention).

Contract: kernel(**inputs) takes FULL unsharded inputs (as in
reference.setup_inputs()), shards the video batch over 8 NeuronCores
(2 videos = 24 frames per core), runs one SPMD Bass program, and returns
the FULL [192, 197, 768] fp32 output.

Math notes (exact algebra, not approximations):
  - qkv = x@(W + lora_b@lora_a).T + in_proj_bias  (LoRA folded on host)
  - v bias is folded through the out projection: attn@(v + 1 b_v^T) @ Wo^T
    = attn@v @ Wo^T + 1 (b_v @ Wo^T)^T, merged with out_proj_bias into one
    rank-1 bias row added via a K=1 matmul.
  - softmax computed without max subtraction (scores here are O(1); exp is
    well inside fp32 range), matching softmax exactly in exact arithmetic.
  - cross-frame attention outputs are normalized after the AV matmul
    (linearity of AV in the attention weights).
Matmuls run in bf16 with fp32 PSUM accumulation.

Performance structure (v3, 612.7us vs 1063us baseline on TRN2):
  - x is pre-transposed and pre-cast to bf16 on the host, fed as per-pair
    tiles xT [128, 6*394] (two seqs side by side); QKV weights are packed
    m-block-major so the first matmul only waits on a 192KB DMA.
  - QKV runs 2 seqs per matmul (N=394) with the weight tile stationary;
    merged exp covers both key chunks of a head in one activation (the
    PSUM zero-region rows are defined via persistent ring tiles).
  - emission is a software-pipelined task queue: per-head attention work of
    pair p-1 (scores -> exp -> AV -> normalize) is interleaved between the
    QKV/V-proj chunks of pair p, with every dependent tensor op placed >=2
    tasks behind its scalar/vector producer so the in-order tensor queue
    never head-blocks (keeps the PE near its top p-state).
  - k^T tiles stay SBUF-resident for the whole video; the cross-frame
    epilogue computes scores transposed ([keys, queries]) so the attention
    probabilities feed the AV matmul directly -- no PE transposes or
    PSUM->SBUF repacks -- with softmax sums from ones-columns in the spilled
    v tiles and normalization applied at extraction.
  - softmax reciprocal rows are broadcast to 64 partitions with the gpsimd
    partition_broadcast DMA (input must be at partition 0 on hardware).
  - v spills / reloads are gpsimd-issued DMAs (25ns dispatch vs ~700ns on
    sync), contiguous in a [keys, 780] layout.
"""

import sys
from collections import deque

sys.path.insert(0, "/opt/trn_rl_repo")

import numpy as np
import ml_dtypes

import concourse.bass as bass
import concourse.mybir as mybir
import concourse.tile as tile
from concourse import bacc
from concourse.bass_utils import run_bass_kernel_spmd
from concourse.masks import make_identity

F32 = mybir.dt.float32
BF16 = mybir.dt.bfloat16
AF = mybir.ActivationFunctionType
MUL = mybir.AluOpType.mult

NCORES = 8
B, F, T, E, H, D, R = 16, 12, 197, 768, 12, 64, 8
NV = B // NCORES          # videos per core = 2
S = NV * F                # seqs per core = 24
NP = S // 2               # seq pairs per core = 12
KT = E // 128             # 6 feature k-tiles
T2 = 2 * T                # 394: two seqs of tokens side by side
G = 8                     # seqs per out-proj group
NG = S // G
GT = G * T                # tokens per group = 1576
SCALE = float(D) ** -0.5
TQ = T + 1                # 198: pair column stride inside ps_o (4B-aligned)
FT = F * T                # keys per video for cross-frame = 2364
NKC = 2 * F               # irregular key chunks per video (128/69 per frame)

_last_results = None  # test harness reads exec_time_ns from here


def _bf(x):
    return np.ascontiguousarray(x.astype(ml_dtypes.bfloat16))


def _f32(x):
    return np.ascontiguousarray(x.astype(np.float32))


def _build(has_qk_bias, has_orow_bias, has_down_bias, has_cls_bias):
    nc = bacc.Bacc("TRN2", target_bir_lowering=False, debug=False,
                   num_devices=NCORES)

    xt_d = nc.declare_dram_parameter("xt", [NP, 128, KT * T2], BF16,
                                     isOutput=False)
    wqk_d = nc.declare_dram_parameter("w_qkt", [2 * KT, 128, KT * 128], BF16,
                                      isOutput=False)
    wv_d = nc.declare_dram_parameter("w_vt", [128, KT * E], BF16,
                                     isOutput=False)
    wo_d = nc.declare_dram_parameter("w_ot", [128, KT * E], BF16,
                                     isOutput=False)
    bqk_d = nc.declare_dram_parameter("b_qk_t", [128, 2 * KT], F32,
                                      isOutput=False)
    brow_d = nc.declare_dram_parameter("bias_row_o", [1, E], BF16,
                                       isOutput=False)
    bcls_d = nc.declare_dram_parameter("b_cls_t", [128, KT], F32,
                                       isOutput=False)
    dwt_d = nc.declare_dram_parameter("down_wt", [128, KT * R], BF16,
                                      isOutput=False)
    bdown_d = nc.declare_dram_parameter("b_down", [R, 1], F32, isOutput=False)
    uwt_d = nc.declare_dram_parameter("up_wt", [R, E], BF16, isOutput=False)

    y_d = nc.declare_dram_parameter("y", [S, T, E], F32, isOutput=True)
    vs_d = nc.dram_tensor("v_scr", [NV, FT, H * (D + 1)], BF16)

    y_flat = y_d.ap().rearrange("a b c -> (a b) c")

    with tile.TileContext(nc) as tc:
        with (
            tc.tile_pool(name="cst", bufs=1) as cst,
            tc.tile_pool(name="sb", bufs=2) as sb,
            tc.tile_pool(name="psmm", bufs=2, space="PSUM") as psmm,
            tc.tile_pool(name="pssc", bufs=1, space="PSUM") as pssc,
            tc.tile_pool(name="psav", bufs=3, space="PSUM") as psav,
            tc.tile_pool(name="psep", bufs=1, space="PSUM") as psep,
        ):
            # ---------------- input prefetch + constants ----------------
            # first two xt pair-tiles go out before the weights so the QKV
            # pipeline can start as soon as wqk lands
            xt_tiles = {}

            def load_xt(p):
                t_ = sb.tile([128, KT * T2], BF16, tag="xt", bufs=4,
                             name=f"xt{p}")
                nc.sync.dma_start(out=t_[:], in_=xt_d[p])
                xt_tiles[p] = t_

            load_xt(0)
            wqkm = []
            qs = [nc.sync, nc.scalar, nc.gpsimd]
            for m in range(2 * KT):
                t_ = cst.tile([128, KT * 128], BF16, tag=f"wqkm{m}",
                              name=f"wqkm{m}")
                qs[m % 3].dma_start(out=t_[:], in_=wqk_d[m])
                wqkm.append(t_)
            wvt = cst.tile([128, KT * E], BF16, tag="wvt")
            nc.gpsimd.dma_start(out=wvt[:], in_=wv_d[:, :])
            load_xt(1)
            wot = cst.tile([128, KT * E], BF16, tag="wot")
            nc.scalar.dma_start(out=wot[:], in_=wo_d[:, :])
            bqk = cst.tile([128, 2 * KT], F32, tag="bqk")
            nc.sync.dma_start(out=bqk[:], in_=bqk_d[:, :])
            brow = cst.tile([1, E], BF16, tag="brow")
            nc.sync.dma_start(out=brow[:], in_=brow_d[:, :])
            bcls = cst.tile([128, KT], F32, tag="bcls")
            nc.sync.dma_start(out=bcls[:], in_=bcls_d[:, :])
            dwtt = cst.tile([128, KT * R], BF16, tag="dwtt")
            nc.sync.dma_start(out=dwtt[:], in_=dwt_d[:, :])
            bdown = cst.tile([R, 1], F32, tag="bdown")
            nc.sync.dma_start(out=bdown[:], in_=bdown_d[:, :])
            uwt = cst.tile([R, E], BF16, tag="uwt")
            nc.sync.dma_start(out=uwt[:], in_=uwt_d[:, :])

            identh = cst.tile([128, 128], BF16, tag="identh")
            make_identity(nc, identh[:])
            identf = cst.tile([128, 128], F32, tag="identf")
            make_identity(nc, identf[:])
            ones_h = cst.tile([97, 128], BF16, tag="ones_h")
            nc.vector.memset(ones_h[:], 1.0)

            # persistent cross-frame state
            q1s = [cst.tile([128, S], BF16, tag=f"q1s{m}", name=f"q1s{m}")
                   for m in range(KT)]
            ocfT = [cst.tile([128, S], BF16, tag=f"ocfT{k}", name=f"ocfT{k}")
                    for k in range(KT)]
            qbd = [[None] * KT for _ in range(NV)]
            ycls = cst.tile([S, E], F32, tag="ycls", name="ycls")

            # Persistent ring buffers: tiles whose pad regions must stay
            # defined across reuses. Using one tile id per slot keeps the
            # race detector's happens-before tracking intact (subtile deps)
            # while the one-time initialization of the pad bytes persists.
            vsl_ring = []
            for i in range(6):
                t_ = cst.tile([128, H * (D + 1)], BF16, tag=f"vsl{i}",
                              name=f"vsl{i}")
                t3 = t_[:].rearrange("p (h d) -> p h d", h=H)
                nc.gpsimd.memset(t3[:, :, D:D + 1], 1.0)
                vsl_ring.append(t_)
            # two persistent PSUM score banks: the merged exp's zero-region
            # rows read bytes only this one-time memset wrote (exp(0)=1,
            # never consumed downstream)
            psc_ring = []
            for i in range(2):
                t_ = pssc.tile([128, 512], F32, tag=f"psc{i}", name=f"psc{i}")
                nc.vector.memset(t_[:], 0.0)
                psc_ring.append(t_)
            cnt = {"psc": 0, "vsl": 0, "smt": 0}

            def next_psc():
                t_ = psc_ring[cnt["psc"] % 2]
                cnt["psc"] += 1
                return t_

            # ---------------- task machinery ----------------
            pend = deque()

            def drain(n):
                for _ in range(n):
                    if pend:
                        pend.popleft()()

            qtiles = {}     # (p, m 0..5) -> q^T tile [128, T2]
            ktiles = {}     # (p, j 0..5) -> k^T tile [128, T2]
            vslabs = {}     # (s, ci) -> vslab tile
            pT_store = {}
            smt_cur = {}
            pair_ctx = {}
            attnTg = {}
            epi_acc = {}
            epi_rcs = {}
            epi_vt = {}
            epi_pl = {}

            # ---------------- per-pair inline emission ----------------
            def emit_pair_qkv(p):
                xt = xt_tiles[p]
                for m in range(2 * KT):
                    ps = psmm.tile([128, 512], F32, tag="pmm", name="psqkv")
                    for k in range(KT):
                        nc.tensor.matmul(ps[:, :T2],
                                         wqkm[m][:, 128 * k:128 * (k + 1)],
                                         xt[:, T2 * k:T2 * (k + 1)],
                                         start=(k == 0), stop=(k == KT - 1))
                    if m < KT:
                        t_ = sb.tile([128, T2], BF16, tag="qt", bufs=12,
                                     name=f"qt{p}_{m}")
                    else:
                        t_ = sb.tile([128, T2], BF16, tag="kt", bufs=54,
                                     name=f"kt{p}_{m - KT}")
                    on_scalar = (m % 3 == 0)
                    if has_qk_bias:
                        if on_scalar:
                            nc.scalar.activation(t_[:], ps[:, :T2], AF.Identity,
                                                 bias=bqk[:, m:m + 1])
                        else:
                            nc.vector.tensor_scalar_add(t_[:], ps[:, :T2],
                                                        bqk[:, m:m + 1])
                    else:
                        if on_scalar:
                            nc.scalar.copy(t_[:], ps[:, :T2])
                        else:
                            nc.vector.tensor_copy(t_[:], ps[:, :T2])
                    if m < KT:
                        qtiles[(p, m)] = t_
                        # CLS queries: col 0 of each seq half
                        src = t_[:].rearrange("p (b c) -> p b c", c=T)[:, :, 0:1]
                        nc.gpsimd.tensor_copy(
                            q1s[m][:, 2 * p:2 * p + 2],
                            src.rearrange("p b c -> p (b c)"))
                    else:
                        ktiles[(p, m - KT)] = t_
                    drain(4)

            def emit_pair_v(p):
                xt = xt_tiles[p]
                for s in (2 * p, 2 * p + 1):
                    v, f = s // F, s % F
                    for ci in range(2):
                        off = 197 * (s % 2) + 128 * ci
                        rows = 128 if ci == 0 else 69
                        psA = psmm.tile([128, 512], F32, tag="pmm", name="psva")
                        psB = psmm.tile([128, 512], F32, tag="pmm", name="psvb")
                        for k in range(KT):
                            lh = xt[:, T2 * k + off:T2 * k + off + rows]
                            nc.tensor.matmul(psA[:rows, :512], lh,
                                             wvt[:, E * k:E * k + 512],
                                             start=(k == 0), stop=(k == KT - 1))
                            nc.tensor.matmul(psB[:rows, :256], lh,
                                             wvt[:, E * k + 512:E * (k + 1)],
                                             start=(k == 0), stop=(k == KT - 1))
                        vt = vsl_ring[cnt["vsl"] % 6]
                        cnt["vsl"] += 1
                        vt3 = vt[:].rearrange("p (h d) -> p h d", h=H)
                        # ones columns persist from the pre-init pass
                        srcA = psA[:rows, :512].rearrange("p (h d) -> p h d",
                                                          h=8)
                        srcB = psB[:rows, :256].rearrange("p (h d) -> p h d",
                                                          h=4)
                        if ci == 0:
                            nc.vector.tensor_copy(vt3[:rows, 0:8, 0:D], srcA)
                            nc.scalar.copy(vt3[:rows, 8:12, 0:D], srcB)
                        else:
                            nc.scalar.copy(vt3[:rows, 0:8, 0:D], srcA)
                            nc.vector.tensor_copy(vt3[:rows, 8:12, 0:D], srcB)
                        nc.gpsimd.dma_start(
                            out=vs_d[v, T * f + 128 * ci:T * f + 128 * ci + rows, :],
                            in_=vt[:rows, :])
                        vslabs[(s, ci)] = vt
                        drain(4)

            # ---------------- local attention tasks ----------------
            def task_sc(s, j, i):
                def go():
                    p, sc = s // 2, 197 * (s % 2)
                    kt_, qt_ = ktiles[(p, j)], qtiles[(p, j)]
                    r0 = 64 * i
                    ps_s = next_psc()
                    nc.tensor.matmul(ps_s[:, 0:T], kt_[r0:r0 + 64, sc:sc + 128],
                                     qt_[r0:r0 + 64, sc:sc + T],
                                     start=True, stop=True)
                    nc.tensor.matmul(ps_s[0:69, T:T2],
                                     kt_[r0:r0 + 64, sc + 128:sc + T],
                                     qt_[r0:r0 + 64, sc:sc + T],
                                     start=True, stop=True)
                    pT = sb.tile([128, T2], BF16, tag="pT", bufs=8,
                                 name=f"pT{s}_{j}_{i}")
                    # rows 69:128 of cols T:T2 are the bank's zero region:
                    # exp(0)=1, never read by the K=69 AV matmul
                    nc.scalar.activation(pT[:], ps_s[:, 0:T2], AF.Exp,
                                         scale=SCALE)
                    pT_store[(s, j, i)] = pT
                return go

            def task_av(s, j):
                def go():
                    g, sg = s // G, s % G
                    ps_o = psav.tile([128, 512], F32, tag="pav", name="ps_o")
                    for i in range(2):
                        h = 2 * j + i
                        pT = pT_store.pop((s, j, i))
                        vs0, vs1 = vslabs[(s, 0)], vslabs[(s, 1)]
                        nc.tensor.matmul(ps_o[:D + 1, TQ * i:TQ * i + T],
                                         vs0[:, (D + 1) * h:(D + 1) * (h + 1)],
                                         pT[:, 0:T], start=True, stop=False)
                        nc.tensor.matmul(ps_o[:D + 1, TQ * i:TQ * i + T],
                                         vs1[0:69, (D + 1) * h:(D + 1) * (h + 1)],
                                         pT[0:69, T:T2], start=False, stop=True)
                    if j % 2 == 0:
                        pair_ctx[s] = []
                    smt = sb.tile([1, T2], F32, tag="smt", bufs=4, name="smt")
                    src = ps_o[D:D + 1, 0:2 * TQ].rearrange(
                        "p (b c) -> p b c", c=TQ)[:, :, 0:T]
                    dst = smt[0:1, :].rearrange("p (b c) -> p b c", c=T)
                    if j % 2 == 0:
                        nc.scalar.copy(dst, src)
                    else:
                        nc.vector.tensor_copy(dst, src)
                    rinv = sb.tile([1, T2], F32, tag="rinv", bufs=4,
                                   name="rinv")
                    nc.vector.reciprocal_approx_fast(rinv[:], smt[:])
                    rb = sb.tile([D, T2], F32, tag="rb", bufs=4, name="rb")
                    nc.gpsimd.partition_broadcast(rb[:], rinv[0:1, :])
                    pair_ctx[s].append((j, rb, ps_o))
                return go

            def task_fin(s):
                def go():
                    g, sg = s // G, s % G
                    gt = attnTg[g]
                    entries = pair_ctx[s]
                    for (jj, rb, ps_o) in entries:
                        for i in range(2):
                            nc.vector.tensor_tensor(
                                out=gt[jj][64 * i:64 * i + 64,
                                           T * sg:T * (sg + 1)],
                                in0=ps_o[0:D, TQ * i:TQ * i + T],
                                in1=rb[:, T * i:T * (i + 1)],
                                op=MUL)
                    pair_ctx[s] = []
                return go

            def attn_tasks(p):
                out = []
                for s in (2 * p, 2 * p + 1):
                    for j in range(H // 2):
                        out.append(task_sc(s, j, 0))
                        out.append(task_sc(s, j, 1))
                        if j >= 1:
                            out.append(task_av(s, j - 1))
                            if j % 2 == 0:
                                out.append(task_fin(s))
                    out.append(task_av(s, H // 2 - 1))
                    out.append(task_fin(s))
                return out

            # ---------------- out-proj tasks ----------------
            def task_group_tile(g, tt):
                def go():
                    gt = attnTg[g]
                    c0 = 128 * tt
                    rows = min(128, GT - c0)
                    psA = psmm.tile([128, 512], F32, tag="pmm", name="psoa")
                    psB = psmm.tile([128, 512], F32, tag="pmm", name="psob")
                    laststop = not has_orow_bias
                    for k in range(KT):
                        lh = gt[k][:, c0:c0 + rows]
                        nc.tensor.matmul(psA[:rows, :512], lh,
                                         wot[:, E * k:E * k + 512],
                                         start=(k == 0),
                                         stop=(k == KT - 1 and laststop))
                        nc.tensor.matmul(psB[:rows, :256], lh,
                                         wot[:, E * k + 512:E * (k + 1)],
                                         start=(k == 0),
                                         stop=(k == KT - 1 and laststop))
                    if has_orow_bias:
                        nc.tensor.matmul(psA[:rows, :512], ones_h[:, :rows],
                                         brow[:, 0:512], start=False, stop=True)
                        nc.tensor.matmul(psB[:rows, :256], ones_h[:, :rows],
                                         brow[:, 512:768], start=False,
                                         stop=True)
                    of = sb.tile([128, E], F32, tag="of", bufs=2, name="of")
                    nc.scalar.copy(of[:rows, 0:512], psA[:rows, :512])
                    nc.vector.tensor_copy(of[:rows, 512:768], psB[:rows, :256])
                    r0 = GT * g + c0
                    s_cls = (r0 + T - 1) // T
                    if T * s_cls < r0 + rows:
                        nc.gpsimd.dma_start(
                            out=ycls[s_cls:s_cls + 1, :],
                            in_=of[T * s_cls - r0:T * s_cls - r0 + 1, :])
                    nc.sync.dma_start(out=y_flat[r0:r0 + rows, :],
                                      in_=of[:rows, :])
                return go

            # ---------------- cross-frame epilogue tasks ----------------
            # Transposed formulation: scores land as [keys, queries] so the
            # attention probabilities feed the AV matmul directly as the
            # stationary operand (no PE transposes / PSUM->SBUF repacks).
            # Three passes of 4 heads (queries at 32-aligned row blocks);
            # softmax sums come from ones-columns in the v tiles and the
            # normalization is applied at extraction (AV is linear in the
            # attention weights).
            def task_qbd(v):
                def go():
                    for m in range(KT):
                        bd = cst.tile([128, 64], BF16, tag=f"qbd{v}_{m}",
                                      name=f"qbd{v}_{m}")
                        nc.gpsimd.memset(bd[:], 0.0)
                        nc.gpsimd.tensor_copy(bd[0:64, 0:F],
                                              q1s[m][0:64, F * v:F * (v + 1)])
                        nc.gpsimd.tensor_copy(bd[64:128, 32:32 + F],
                                              q1s[m][64:128, F * v:F * (v + 1)])
                        qbd[v][m] = bd
                    _issue_vt(v, 0, 0)
                    _issue_vt(v, 0, 1)
                return go

            def _chunk_geo(c):
                f2, ci = c // 2, c % 2
                return (T * f2 + 128 * ci, 128 if ci == 0 else 69, f2, ci)

            def _issue_vt(v, pp, c):
                c0, rows, f2, ci = _chunk_geo(c)
                vt_ = sb.tile([128, 4 * (D + 1)], BF16, tag="vte", bufs=8,
                              name="vte")
                nc.gpsimd.dma_start(out=vt_[:rows, :],
                                    in_=vs_d[v, c0:c0 + rows,
                                             4 * (D + 1) * pp:
                                             4 * (D + 1) * (pp + 1)])
                epi_vt[(v, pp, c)] = vt_

            def task_epi_sc(v, pp, c):
                def go():
                    c0, rows, f2, ci = _chunk_geo(c)
                    if c + 2 < NKC:
                        _issue_vt(v, pp, c + 2)
                    elif pp < 2:
                        _issue_vt(v, pp + 1, c + 2 - NKC)
                    if c == 0:
                        epi_acc[v] = psep.tile([128, 4 * (D + 1)], F32,
                                               tag="peav", name="peav")
                    ps = next_psc()
                    kt_off = 197 * (f2 % 2) + 128 * ci
                    for pl in range(2):
                        jp = 2 * pp + pl
                        kt_ = ktiles[(6 * v + f2 // 2, jp)]
                        nc.tensor.matmul(
                            ps[:rows, 64 * pl:64 * pl + 64],
                            kt_[:, kt_off:kt_off + rows], qbd[v][jp][:],
                            start=True, stop=True)
                    pl_t = sb.tile([128, 128], BF16, tag="plT", bufs=8,
                                   name="plT")
                    nc.scalar.activation(pl_t[:rows, :], ps[:rows, 0:128],
                                         AF.Exp, scale=SCALE)
                    epi_pl[(v, pp, c)] = pl_t
                return go

            def task_epi_av(v, pp, c):
                def go():
                    c0, rows, f2, ci = _chunk_geo(c)
                    pl_t = epi_pl.pop((v, pp, c))
                    vt_ = epi_vt.pop((v, pp, c))
                    nc.tensor.matmul(epi_acc[v][0:128, 0:4 * (D + 1)],
                                     pl_t[:rows, :], vt_[:rows, :],
                                     start=(c == 0), stop=(c == NKC - 1))
                return go

            def task_esum(v, pp):
                def go():
                    acc = epi_acc[v]
                    sg_ = sb.tile([128, 4], F32, tag="esum", bufs=2,
                                  name="esum")
                    nc.vector.tensor_copy(
                        sg_[:], acc[:, :].rearrange("p (h d) -> p h d",
                                                    h=4)[:, :, D:D + 1]
                        .rearrange("p h d -> p (h d)"))
                    rcs = sb.tile([128, 4], F32, tag="ercs", bufs=2,
                                  name="ercs")
                    nc.vector.reciprocal(rcs[:], sg_[:])
                    epi_rcs[v] = rcs
                return go

            def task_eext(v, pp, hl):
                def go():
                    acc = epi_acc[v]
                    rcs = epi_rcs[v]
                    hg = 4 * pp + hl
                    r0 = 64 * (hl // 2) + 32 * (hl % 2)
                    st = sb.tile([F, D], BF16, tag="st", bufs=4, name="st")
                    nc.vector.tensor_scalar_mul(
                        st[:], acc[r0:r0 + F, 65 * hl:65 * hl + D],
                        rcs[r0:r0 + F, hl:hl + 1])
                    pst = next_psc()[:].bitcast(BF16)
                    nc.tensor.transpose(pst[:D, 0:F], st[:], identh[:F, :F])
                    nc.scalar.copy(
                        ocfT[hg // 2][64 * (hg % 2):64 * (hg % 2) + D,
                                      F * v:F * (v + 1)],
                        pst[:D, 0:F])
                return go

            def epi_tasks(v):
                tasks = [task_qbd(v)]
                for pp in range(3):
                    for c in range(NKC):
                        tasks.append(task_epi_sc(v, pp, c))
                        if c >= 3:
                            tasks.append(task_epi_av(v, pp, c - 3))
                    for c in range(NKC - 3, NKC):
                        tasks.append(task_epi_av(v, pp, c))
                    tasks.append(task_esum(v, pp))
                    tasks += [task_eext(v, pp, hl) for hl in range(4)]
                return tasks

            # ---------------- finale: CLS adapter + cross-frame out-proj ----
            def emit_finale():
                yclsh = sb.tile([S, E], BF16, bufs=1, tag="yclsh",
                                name="yclsh")
                nc.vector.tensor_copy(yclsh[:], ycls[:])
                yclsT = []
                for k in range(KT):
                    pst = next_psc()[:].bitcast(BF16)
                    nc.tensor.transpose(pst[:, :S], yclsh[:, 128 * k:128 * (k + 1)],
                                        identh[:S, :S])
                    t_ = sb.tile([128, S], BF16, tag="yclsT", bufs=6,
                                 name=f"yclsT{k}")
                    nc.scalar.copy(t_[:], pst[:, :S])
                    yclsT.append(t_)
                ps8 = psav.tile([128, 512], F32, tag="pav", name="ps8")
                for k in range(KT):
                    nc.tensor.matmul(ps8[:R, :S], dwtt[:, R * k:R * (k + 1)],
                                     yclsT[k][:],
                                     start=(k == 0), stop=(k == KT - 1))
                z = sb.tile([R, S], F32, tag="z8", name="z8")
                if has_down_bias:
                    nc.scalar.activation(z[:], ps8[:R, :S], AF.Identity,
                                         bias=bdown[:, 0:1])
                else:
                    nc.scalar.copy(z[:], ps8[:R, :S])
                en = sb.tile([R, S], F32, tag="sg8", name="sg8")
                nc.scalar.activation(en[:], z[:], AF.Exp, scale=-1.702)
                nc.vector.tensor_scalar_add(en[:], en[:], 1.0)
                rec = sb.tile([R, S], F32, tag="rec8", name="rec8")
                nc.vector.reciprocal_approx_fast(rec[:], en[:])
                gq = sb.tile([R, S], BF16, tag="gq8", name="gq8")
                nc.vector.tensor_tensor(out=gq[:], in0=z[:], in1=rec[:],
                                        op=MUL)
                cn = sb.tile([S, E], F32, tag="cn", bufs=1, name="cn")
                for m in range(KT):
                    ps = psmm.tile([128, 512], F32, tag="pmm", name="psf")
                    nc.tensor.matmul(ps[:, :S], uwt[:, 128 * m:128 * (m + 1)],
                                     gq[:], start=True, stop=False)
                    for k in range(KT):
                        nc.tensor.matmul(ps[:, :S],
                                         wot[:, E * k + 128 * m:
                                              E * k + 128 * (m + 1)],
                                         ocfT[k][:], start=False,
                                         stop=(k == KT - 1))
                    cnT = sb.tile([128, S], F32, tag="cnT", bufs=2, name="cnT")
                    if has_cls_bias:
                        nc.scalar.activation(cnT[:], ps[:, :S], AF.Identity,
                                             bias=bcls[:, m:m + 1])
                    else:
                        if m % 2 == 0:
                            nc.scalar.copy(cnT[:], ps[:, :S])
                        else:
                            nc.vector.tensor_copy(cnT[:], ps[:, :S])
                    pst = next_psc()
                    nc.tensor.transpose(pst[:S, 0:128], cnT[:], identf[:, :])
                    if m % 2 == 0:
                        nc.vector.tensor_copy(cn[:, 128 * m:128 * (m + 1)],
                                              pst[:S, 0:128])
                    else:
                        nc.scalar.copy(cn[:, 128 * m:128 * (m + 1)],
                                       pst[:S, 0:128])
                nc.sync.dma_start(out=y_d[:, 0, :], in_=cn[:])

            # ---------------- emission schedule ----------------
            load_xt(2)
            epi0 = None
            ntt = (GT + 127) // 128
            for p in range(NP):
                g = (2 * p) // G
                if (2 * p) % G == 0:
                    attnTg[g] = [sb.tile([128, GT], BF16, tag="gt", bufs=12,
                                         name=f"gt{g}_{k}")
                                 for k in range(KT)]
                if p + 3 < NP:
                    load_xt(p + 3)
                emit_pair_qkv(p)
                emit_pair_v(p)
                if p == 11:
                    # tail: interleave the last pair's attention, the last
                    # group's out-proj, and the second video's epilogue so
                    # exp- and DMA-bound chains hide behind tensor-dense work
                    at = attn_tasks(11)
                    op_ = [task_group_tile(2, tt) for tt in range(ntt)]
                    ep = epi_tasks(1)
                    ia = ib = 0
                    while ia < len(at) or ib < len(ep):
                        if ia < len(at):
                            pend.append(at[ia]); ia += 1
                        for _ in range(2):
                            if ib < len(ep):
                                pend.append(ep[ib]); ib += 1
                        if ia >= len(at) and op_:
                            pend.append(op_.pop(0))
                    pend.extend(op_)
                else:
                    pend.extend(attn_tasks(p))
                    if p % 4 == 3:
                        for tt in range(ntt):
                            pend.append(task_group_tile(g, tt))
                if p == 5:
                    epi0 = epi_tasks(0)
                if epi0:
                    pend.extend(epi0[:45])
                    epi0 = epi0[45:]
                # bound the backlog: stale tasks past ~1 pair break the ring
                # buffers' reuse-distance assumptions
                while len(pend) > 36:
                    drain(1)
            drain(len(pend))
            emit_finale()

    nc.finalize()
    return nc


def _preprocess(x, in_proj_weight, in_proj_bias, out_proj_weight,
                out_proj_bias, lora_a, lora_b, down_w, down_b, up_w, up_b):
    w_comb = in_proj_weight.astype(np.float64) + \
        lora_b.astype(np.float64) @ lora_a.astype(np.float64)
    w_comb = w_comb.astype(np.float32)
    b_v = in_proj_bias[2 * E:3 * E].astype(np.float32)
    bias_row = out_proj_bias.astype(np.float32) + \
        b_v @ out_proj_weight.T.astype(np.float32)
    b_cls = bias_row + up_b.astype(np.float32)
    def _pack(w):  # [E, C] -> [128, KT*C] with k-major columns
        kt = w.reshape(KT, 128, -1)
        return _bf(kt.transpose(1, 0, 2).reshape(128, -1))

    p = {
        "w_qkt": _bf(np.ascontiguousarray(
            w_comb[0:2 * E].T.reshape(KT, 128, 2 * KT, 128)
            .transpose(2, 1, 0, 3).reshape(2 * KT, 128, KT * 128))),
        "w_vt": _pack(np.ascontiguousarray(w_comb[2 * E:3 * E].T)),
        "w_ot": _pack(np.ascontiguousarray(out_proj_weight.T)),
        "b_qk_t": _f32(in_proj_bias[0:2 * E].reshape(2 * KT, 128).T),
        "bias_row_o": _bf(bias_row.reshape(1, E)),
        "b_cls_t": _f32(b_cls.reshape(KT, 128).T),
        "down_wt": _pack(np.ascontiguousarray(down_w.T)),
        "b_down": _f32(down_b.reshape(R, 1)),
        "up_wt": _bf(up_w.T),
    }
    flags = (
        bool(np.any(in_proj_bias[0:2 * E])),
        bool(np.any(bias_row)),
        bool(np.any(down_b)),
        bool(np.any(b_cls)),
    )
    # xt per core: [NP, 128, KT*T2] with cols k-major, two seqs side by side
    xts = []
    xb = x.astype(ml_dtypes.bfloat16)
    for c in range(NCORES):
        xc = xb[S * c:S * (c + 1)]            # [S, T, E]
        xt = np.empty((NP, 128, KT * T2), dtype=ml_dtypes.bfloat16)
        for pi in range(NP):
            a = xc[2 * pi].T.reshape(KT, 128, T)      # [KT, 128, T]
            b = xc[2 * pi + 1].T.reshape(KT, 128, T)
            blk = np.concatenate([a, b], axis=2)      # [KT, 128, T2]
            xt[pi] = blk.transpose(1, 0, 2).reshape(128, KT * T2)
        xts.append(np.ascontiguousarray(xt))
    return p, flags, xts


def kernel(x, in_proj_weight, in_proj_bias, out_proj_weight, out_proj_bias,
           lora_a, lora_b, down_w, down_b, up_w, up_b,
           b, n_f, token_len, d_v):
    global _last_results
    x = np.asarray(x, dtype=np.float32)
    assert x.shape == (B * F, T, E), x.shape
    params, flags, xts = _preprocess(
        x, np.asarray(in_proj_weight), np.asarray(in_proj_bias),
        np.asarray(out_proj_weight), np.asarray(out_proj_bias),
        np.asarray(lora_a), np.asarray(lora_b),
        np.asarray(down_w), np.asarray(down_b),
        np.asarray(up_w), np.asarray(up_b))

    nc = _build(*flags)

    in_maps = []
    for c in range(NCORES):
        m = dict(params)
        m["xt"] = xts[c]
        in_maps.append(m)

    res = run_bass_kernel_spmd(nc, in_maps, list(range(NCORES)))
    _last_results = res
    out = np.concatenate([res.results[c]["y"] for c in range(NCORES)], axis=0)
    return out.astype(np.float32)


# revision 53
# speedup vs baseline: 1.2124x; 1.0042x over previous
"""Trainium2 Bass kernel for the LoRA-QKV + per-frame local attention +
cross-frame CLS attention + adapter module (nn_Attention sparse_attention).

Contract: kernel(**inputs) takes FULL unsharded inputs (as in
reference.setup_inputs()), shards the video batch over 8 NeuronCores
(2 videos = 24 frames per core), runs one SPMD Bass program, and returns
the FULL [192, 197, 768] fp32 output.

Math notes (exact algebra, not approximations):
  - qkv = x@(W + lora_b@lora_a).T + in_proj_bias  (LoRA folded on host)
  - v bias is folded through the out projection: attn@(v + 1 b_v^T) @ Wo^T
    = attn@v @ Wo^T + 1 (b_v @ Wo^T)^T, merged with out_proj_bias into one
    rank-1 bias row added via a K=1 matmul.
  - softmax computed without max subtraction (scores here are O(1); exp is
    well inside fp32 range), matching softmax exactly in exact arithmetic.
  - cross-frame attention outputs are normalized after the AV matmul
    (linearity of AV in the attention weights).
Matmuls run in bf16 with fp32 PSUM accumulation.

Performance structure (v3, 612.7us vs 1063us baseline on TRN2):
  - x is pre-transposed and pre-cast to bf16 on the host, fed as per-pair
    tiles xT [128, 6*394] (two seqs side by side); QKV weights are packed
    m-block-major so the first matmul only waits on a 192KB DMA.
  - QKV runs 2 seqs per matmul (N=394) with the weight tile stationary;
    merged exp covers both key chunks of a head in one activation (the
    PSUM zero-region rows are defined via persistent ring tiles).
  - emission is a software-pipelined task queue: per-head attention work of
    pair p-1 (scores -> exp -> AV -> normalize) is interleaved between the
    QKV/V-proj chunks of pair p, with every dependent tensor op placed >=2
    tasks behind its scalar/vector producer so the in-order tensor queue
    never head-blocks (keeps the PE near its top p-state).
  - k^T tiles stay SBUF-resident for the whole video; the cross-frame
    epilogue computes scores transposed ([keys, queries]) so the attention
    probabilities feed the AV matmul directly -- no PE transposes or
    PSUM->SBUF repacks -- with softmax sums from ones-columns in the spilled
    v tiles and normalization applied at extraction.
  - softmax reciprocal rows are broadcast to 64 partitions with the gpsimd
    partition_broadcast DMA (input must be at partition 0 on hardware).
  - v spills / reloads are gpsimd-issued DMAs (25ns dispatch vs ~700ns on
    sync), contiguous in a [keys, 780] layout.
"""

import sys
from collections import deque

sys.path.insert(0, "/opt/trn_rl_repo")

import numpy as np
import ml_dtypes

import concourse.bass as bass
import concourse.mybir as mybir
import concourse.tile as tile
from concourse import bacc
from concourse.bass_utils import run_bass_kernel_spmd
from concourse.masks import make_identity

F32 = mybir.dt.float32
BF16 = mybir.dt.bfloat16
AF = mybir.ActivationFunctionType
MUL = mybir.AluOpType.mult

NCORES = 8
B, F, T, E, H, D, R = 16, 12, 197, 768, 12, 64, 8
NV = B // NCORES          # videos per core = 2
S = NV * F                # seqs per core = 24
NP = S // 2               # seq pairs per core = 12
KT = E // 128             # 6 feature k-tiles
T2 = 2 * T                # 394: two seqs of tokens side by side
G = 8                     # seqs per out-proj group
NG = S // G
GT = G * T                # tokens per group = 1576
SCALE = float(D) ** -0.5
TQ = T + 1                # 198: pair column stride inside ps_o (4B-aligned)
FT = F * T                # keys per video for cross-frame = 2364
NKC = 2 * F               # irregular key chunks per video (128/69 per frame)

_last_results = None  # test harness reads exec_time_ns from here


def _bf(x):
    return np.ascontiguousarray(x.astype(ml_dtypes.bfloat16))


def _f32(x):
    return np.ascontiguousarray(x.astype(np.float32))


def _build(has_qk_bias, has_orow_bias, has_down_bias, has_cls_bias):
    nc = bacc.Bacc("TRN2", target_bir_lowering=False, debug=False,
                   num_devices=NCORES)

    xt_d = nc.declare_dram_parameter("xt", [NP, 128, KT * T2], BF16,
                                     isOutput=False)
    wqk_d = nc.declare_dram_parameter("w_qkt", [2 * KT, 128, KT * 128], BF16,
                                      isOutput=False)
    wv_d = nc.declare_dram_parameter("w_vt", [128, KT * E], BF16,
                                     isOutput=False)
    wo_d = nc.declare_dram_parameter("w_ot", [128, KT * E], BF16,
                                     isOutput=False)
    bqk_d = nc.declare_dram_parameter("b_qk_t", [128, 2 * KT], F32,
                                      isOutput=False)
    brow_d = nc.declare_dram_parameter("bias_row_o", [1, E], BF16,
                                       isOutput=False)
    bcls_d = nc.declare_dram_parameter("b_cls_t", [128, KT], F32,
                                       isOutput=False)
    dwt_d = nc.declare_dram_parameter("down_wt", [128, KT * R], BF16,
                                      isOutput=False)
    bdown_d = nc.declare_dram_parameter("b_down", [R, 1], F32, isOutput=False)
    uwt_d = nc.declare_dram_parameter("up_wt", [R, E], BF16, isOutput=False)

    y_d = nc.declare_dram_parameter("y", [S, T, E], F32, isOutput=True)
    vs_d = nc.dram_tensor("v_scr", [NV, FT, H * (D + 1)], BF16)

    y_flat = y_d.ap().rearrange("a b c -> (a b) c")

    with tile.TileContext(nc) as tc:
        with (
            tc.tile_pool(name="cst", bufs=1) as cst,
            tc.tile_pool(name="sb", bufs=2) as sb,
            tc.tile_pool(name="psmm", bufs=2, space="PSUM") as psmm,
            tc.tile_pool(name="pssc", bufs=1, space="PSUM") as pssc,
            tc.tile_pool(name="psav", bufs=3, space="PSUM") as psav,
            tc.tile_pool(name="psep", bufs=1, space="PSUM") as psep,
        ):
            # ---------------- input prefetch + constants ----------------
            # first two xt pair-tiles go out before the weights so the QKV
            # pipeline can start as soon as wqk lands
            xt_tiles = {}

            def load_xt(p):
                t_ = sb.tile([128, KT * T2], BF16, tag="xt", bufs=4,
                             name=f"xt{p}")
                # pair 0 rides the gpsimd queue so its transfer overlaps the
                # q-half weight DMAs on sync (first matmul needs both)
                eng = nc.gpsimd if p == 0 else nc.sync
                eng.dma_start(out=t_[:], in_=xt_d[p])
                xt_tiles[p] = t_

            load_xt(0)
            wqkm = []
            qs = [nc.sync, nc.scalar, nc.gpsimd]
            for m in range(2 * KT):
                t_ = cst.tile([128, KT * 128], BF16, tag=f"wqkm{m}",
                              name=f"wqkm{m}")
                qs[m % 3].dma_start(out=t_[:], in_=wqk_d[m])
                wqkm.append(t_)
            wvt = cst.tile([128, KT * E], BF16, tag="wvt")
            nc.gpsimd.dma_start(out=wvt[:], in_=wv_d[:, :])
            load_xt(1)
            wot = cst.tile([128, KT * E], BF16, tag="wot")
            nc.scalar.dma_start(out=wot[:], in_=wo_d[:, :])
            bqk = cst.tile([128, 2 * KT], F32, tag="bqk")
            nc.sync.dma_start(out=bqk[:], in_=bqk_d[:, :])
            brow = cst.tile([1, E], BF16, tag="brow")
            nc.sync.dma_start(out=brow[:], in_=brow_d[:, :])
            bcls = cst.tile([128, KT], F32, tag="bcls")
            nc.sync.dma_start(out=bcls[:], in_=bcls_d[:, :])
            dwtt = cst.tile([128, KT * R], BF16, tag="dwtt")
            nc.sync.dma_start(out=dwtt[:], in_=dwt_d[:, :])
            bdown = cst.tile([R, 1], F32, tag="bdown")
            nc.sync.dma_start(out=bdown[:], in_=bdown_d[:, :])
            uwt = cst.tile([R, E], BF16, tag="uwt")
            nc.sync.dma_start(out=uwt[:], in_=uwt_d[:, :])

            identh = cst.tile([128, 128], BF16, tag="identh")
            make_identity(nc, identh[:])
            identf = cst.tile([128, 128], F32, tag="identf")
            make_identity(nc, identf[:])
            ones_h = cst.tile([97, 128], BF16, tag="ones_h")
            nc.vector.memset(ones_h[:], 1.0)

            # persistent cross-frame state
            q1s = [cst.tile([128, S], BF16, tag=f"q1s{m}", name=f"q1s{m}")
                   for m in range(KT)]
            ocfT = [cst.tile([128, S], BF16, tag=f"ocfT{k}", name=f"ocfT{k}")
                    for k in range(KT)]
            qbd = [[None] * KT for _ in range(NV)]
            ycls = cst.tile([S, E], F32, tag="ycls", name="ycls")

            # Persistent ring buffers: tiles whose pad regions must stay
            # defined across reuses. Using one tile id per slot keeps the
            # race detector's happens-before tracking intact (subtile deps)
            # while the one-time initialization of the pad bytes persists.
            vsl_ring = []
            for i in range(6):
                t_ = cst.tile([128, H * (D + 1)], BF16, tag=f"vsl{i}",
                              name=f"vsl{i}")
                t3 = t_[:].rearrange("p (h d) -> p h d", h=H)
                nc.gpsimd.memset(t3[:, :, D:D + 1], 1.0)
                vsl_ring.append(t_)
            # two persistent PSUM score banks: the merged exp's zero-region
            # rows read bytes only this one-time memset wrote (exp(0)=1,
            # never consumed downstream)
            psc_ring = []
            for i in range(2):
                t_ = pssc.tile([128, 512], F32, tag=f"psc{i}", name=f"psc{i}")
                nc.vector.memset(t_[:], 0.0)
                psc_ring.append(t_)
            cnt = {"psc": 0, "vsl": 0, "smt": 0}

            def next_psc():
                t_ = psc_ring[cnt["psc"] % 2]
                cnt["psc"] += 1
                return t_

            # ---------------- task machinery ----------------
            pend = deque()

            def drain(n):
                for _ in range(n):
                    if pend:
                        pend.popleft()()

            qtiles = {}     # (p, m 0..5) -> q^T tile [128, T2]
            ktiles = {}     # (p, j 0..5) -> k^T tile [128, T2]
            vslabs = {}     # (s, ci) -> vslab tile
            pT_store = {}
            smt_cur = {}
            pair_ctx = {}
            attnTg = {}
            epi_acc = {}
            epi_rcs = {}
            epi_vt = {}
            epi_pl = {}

            # ---------------- per-pair inline emission ----------------
            def emit_pair_qkv(p):
                xt = xt_tiles[p]
                for m in range(2 * KT):
                    ps = psmm.tile([128, 512], F32, tag="pmm", name="psqkv")
                    for k in range(KT):
                        nc.tensor.matmul(ps[:, :T2],
                                         wqkm[m][:, 128 * k:128 * (k + 1)],
                                         xt[:, T2 * k:T2 * (k + 1)],
                                         start=(k == 0), stop=(k == KT - 1))
                    if m < KT:
                        t_ = sb.tile([128, T2], BF16, tag="qt", bufs=12,
                                     name=f"qt{p}_{m}")
                    else:
                        t_ = sb.tile([128, T2], BF16, tag="kt", bufs=54,
                                     name=f"kt{p}_{m - KT}")
                    on_scalar = (m % 3 == 0)
                    if has_qk_bias:
                        if on_scalar:
                            nc.scalar.activation(t_[:], ps[:, :T2], AF.Identity,
                                                 bias=bqk[:, m:m + 1])
                        else:
                            nc.vector.tensor_scalar_add(t_[:], ps[:, :T2],
                                                        bqk[:, m:m + 1])
                    else:
                        if on_scalar:
                            nc.scalar.copy(t_[:], ps[:, :T2])
                        else:
                            nc.vector.tensor_copy(t_[:], ps[:, :T2])
                    if m < KT:
                        qtiles[(p, m)] = t_
                        # CLS queries: col 0 of each seq half
                        src = t_[:].rearrange("p (b c) -> p b c", c=T)[:, :, 0:1]
                        nc.gpsimd.tensor_copy(
                            q1s[m][:, 2 * p:2 * p + 2],
                            src.rearrange("p b c -> p (b c)"))
                    else:
                        ktiles[(p, m - KT)] = t_
                    drain(4)

            def emit_pair_v(p):
                xt = xt_tiles[p]
                for s in (2 * p, 2 * p + 1):
                    v, f = s // F, s % F
                    for ci in range(2):
                        off = 197 * (s % 2) + 128 * ci
                        rows = 128 if ci == 0 else 69
                        psA = psmm.tile([128, 512], F32, tag="pmm", name="psva")
                        psB = psmm.tile([128, 512], F32, tag="pmm", name="psvb")
                        for k in range(KT):
                            lh = xt[:, T2 * k + off:T2 * k + off + rows]
                            nc.tensor.matmul(psA[:rows, :512], lh,
                                             wvt[:, E * k:E * k + 512],
                                             start=(k == 0), stop=(k == KT - 1))
                            nc.tensor.matmul(psB[:rows, :256], lh,
                                             wvt[:, E * k + 512:E * (k + 1)],
                                             start=(k == 0), stop=(k == KT - 1))
                        vt = vsl_ring[cnt["vsl"] % 6]
                        cnt["vsl"] += 1
                        vt3 = vt[:].rearrange("p (h d) -> p h d", h=H)
                        # ones columns persist from the pre-init pass
                        srcA = psA[:rows, :512].rearrange("p (h d) -> p h d",
                                                          h=8)
                        srcB = psB[:rows, :256].rearrange("p (h d) -> p h d",
                                                          h=4)
                        if ci == 0:
                            nc.vector.tensor_copy(vt3[:rows, 0:8, 0:D], srcA)
                            nc.scalar.copy(vt3[:rows, 8:12, 0:D], srcB)
                        else:
                            nc.scalar.copy(vt3[:rows, 0:8, 0:D], srcA)
                            nc.vector.tensor_copy(vt3[:rows, 8:12, 0:D], srcB)
                        nc.gpsimd.dma_start(
                            out=vs_d[v, T * f + 128 * ci:T * f + 128 * ci + rows, :],
                            in_=vt[:rows, :])
                        vslabs[(s, ci)] = vt
                        drain(4)

            # ---------------- local attention tasks ----------------
            def task_sc(s, j, i):
                def go():
                    p, sc = s // 2, 197 * (s % 2)
                    kt_, qt_ = ktiles[(p, j)], qtiles[(p, j)]
                    r0 = 64 * i
                    ps_s = next_psc()
                    nc.tensor.matmul(ps_s[:, 0:T], kt_[r0:r0 + 64, sc:sc + 128],
                                     qt_[r0:r0 + 64, sc:sc + T],
                                     start=True, stop=True)
                    nc.tensor.matmul(ps_s[0:69, T:T2],
                                     kt_[r0:r0 + 64, sc + 128:sc + T],
                                     qt_[r0:r0 + 64, sc:sc + T],
                                     start=True, stop=True)
                    pT = sb.tile([128, T2], BF16, tag="pT", bufs=8,
                                 name=f"pT{s}_{j}_{i}")
                    # rows 69:128 of cols T:T2 are the bank's zero region:
                    # exp(0)=1, never read by the K=69 AV matmul
                    nc.scalar.activation(pT[:], ps_s[:, 0:T2], AF.Exp,
                                         scale=SCALE)
                    pT_store[(s, j, i)] = pT
                return go

            def task_av(s, j):
                def go():
                    g, sg = s // G, s % G
                    ps_o = psav.tile([128, 512], F32, tag="pav", name="ps_o")
                    for i in range(2):
                        h = 2 * j + i
                        pT = pT_store.pop((s, j, i))
                        vs0, vs1 = vslabs[(s, 0)], vslabs[(s, 1)]
                        nc.tensor.matmul(ps_o[:D + 1, TQ * i:TQ * i + T],
                                         vs0[:, (D + 1) * h:(D + 1) * (h + 1)],
                                         pT[:, 0:T], start=True, stop=False)
                        nc.tensor.matmul(ps_o[:D + 1, TQ * i:TQ * i + T],
                                         vs1[0:69, (D + 1) * h:(D + 1) * (h + 1)],
                                         pT[0:69, T:T2], start=False, stop=True)
                    if j % 2 == 0:
                        pair_ctx[s] = []
                    smt = sb.tile([1, T2], F32, tag="smt", bufs=4, name="smt")
                    src = ps_o[D:D + 1, 0:2 * TQ].rearrange(
                        "p (b c) -> p b c", c=TQ)[:, :, 0:T]
                    dst = smt[0:1, :].rearrange("p (b c) -> p b c", c=T)
                    if j % 2 == 0:
                        nc.scalar.copy(dst, src)
                    else:
                        nc.vector.tensor_copy(dst, src)
                    rinv = sb.tile([1, T2], F32, tag="rinv", bufs=4,
                                   name="rinv")
                    nc.vector.reciprocal_approx_fast(rinv[:], smt[:])
                    rb = sb.tile([D, T2], F32, tag="rb", bufs=4, name="rb")
                    nc.gpsimd.partition_broadcast(rb[:], rinv[0:1, :])
                    pair_ctx[s].append((j, rb, ps_o))
                return go

            def task_fin(s):
                def go():
                    g, sg = s // G, s % G
                    gt = attnTg[g]
                    entries = pair_ctx[s]
                    for (jj, rb, ps_o) in entries:
                        for i in range(2):
                            nc.vector.tensor_tensor(
                                out=gt[jj][64 * i:64 * i + 64,
                                           T * sg:T * (sg + 1)],
                                in0=ps_o[0:D, TQ * i:TQ * i + T],
                                in1=rb[:, T * i:T * (i + 1)],
                                op=MUL)
                    pair_ctx[s] = []
                return go

            def attn_tasks(p):
                out = []
                for s in (2 * p, 2 * p + 1):
                    for j in range(H // 2):
                        out.append(task_sc(s, j, 0))
                        out.append(task_sc(s, j, 1))
                        if j >= 1:
                            out.append(task_av(s, j - 1))
                            if j % 2 == 0:
                                out.append(task_fin(s))
                    out.append(task_av(s, H // 2 - 1))
                    out.append(task_fin(s))
                return out

            # ---------------- out-proj tasks ----------------
            def task_group_tile(g, tt):
                def go():
                    gt = attnTg[g]
                    c0 = 128 * tt
                    rows = min(128, GT - c0)
                    psA = psmm.tile([128, 512], F32, tag="pmm", name="psoa")
                    psB = psmm.tile([128, 512], F32, tag="pmm", name="psob")
                    laststop = not has_orow_bias
                    for k in range(KT):
                        lh = gt[k][:, c0:c0 + rows]
                        nc.tensor.matmul(psA[:rows, :512], lh,
                                         wot[:, E * k:E * k + 512],
                                         start=(k == 0),
                                         stop=(k == KT - 1 and laststop))
                        nc.tensor.matmul(psB[:rows, :256], lh,
                                         wot[:, E * k + 512:E * (k + 1)],
                                         start=(k == 0),
                                         stop=(k == KT - 1 and laststop))
                    if has_orow_bias:
                        nc.tensor.matmul(psA[:rows, :512], ones_h[:, :rows],
                                         brow[:, 0:512], start=False, stop=True)
                        nc.tensor.matmul(psB[:rows, :256], ones_h[:, :rows],
                                         brow[:, 512:768], start=False,
                                         stop=True)
                    of = sb.tile([128, E], F32, tag="of", bufs=2, name="of")
                    nc.scalar.copy(of[:rows, 0:512], psA[:rows, :512])
                    nc.vector.tensor_copy(of[:rows, 512:768], psB[:rows, :256])
                    r0 = GT * g + c0
                    s_cls = (r0 + T - 1) // T
                    if T * s_cls < r0 + rows:
                        nc.gpsimd.dma_start(
                            out=ycls[s_cls:s_cls + 1, :],
                            in_=of[T * s_cls - r0:T * s_cls - r0 + 1, :])
                    nc.sync.dma_start(out=y_flat[r0:r0 + rows, :],
                                      in_=of[:rows, :])
                return go

            # ---------------- cross-frame epilogue tasks ----------------
            # Transposed formulation: scores land as [keys, queries] so the
            # attention probabilities feed the AV matmul directly as the
            # stationary operand (no PE transposes / PSUM->SBUF repacks).
            # Three passes of 4 heads (queries at 32-aligned row blocks);
            # softmax sums come from ones-columns in the v tiles and the
            # normalization is applied at extraction (AV is linear in the
            # attention weights).
            def task_qbd(v):
                def go():
                    for m in range(KT):
                        bd = cst.tile([128, 64], BF16, tag=f"qbd{v}_{m}",
                                      name=f"qbd{v}_{m}")
                        nc.gpsimd.memset(bd[:], 0.0)
                        nc.gpsimd.tensor_copy(bd[0:64, 0:F],
                                              q1s[m][0:64, F * v:F * (v + 1)])
                        nc.gpsimd.tensor_copy(bd[64:128, 32:32 + F],
                                              q1s[m][64:128, F * v:F * (v + 1)])
                        qbd[v][m] = bd
                    _issue_vt(v, 0, 0)
                    _issue_vt(v, 0, 1)
                return go

            def _chunk_geo(c):
                f2, ci = c // 2, c % 2
                return (T * f2 + 128 * ci, 128 if ci == 0 else 69, f2, ci)

            def _issue_vt(v, pp, c):
                c0, rows, f2, ci = _chunk_geo(c)
                vt_ = sb.tile([128, 4 * (D + 1)], BF16, tag="vte", bufs=8,
                              name="vte")
                nc.gpsimd.dma_start(out=vt_[:rows, :],
                                    in_=vs_d[v, c0:c0 + rows,
                                             4 * (D + 1) * pp:
                                             4 * (D + 1) * (pp + 1)])
                epi_vt[(v, pp, c)] = vt_

            def task_epi_sc(v, pp, c):
                def go():
                    c0, rows, f2, ci = _chunk_geo(c)
                    if c + 2 < NKC:
                        _issue_vt(v, pp, c + 2)
                    elif pp < 2:
                        _issue_vt(v, pp + 1, c + 2 - NKC)
                    if c == 0:
                        epi_acc[v] = psep.tile([128, 4 * (D + 1)], F32,
                                               tag="peav", name="peav")
                    ps = next_psc()
                    kt_off = 197 * (f2 % 2) + 128 * ci
                    for pl in range(2):
                        jp = 2 * pp + pl
                        kt_ = ktiles[(6 * v + f2 // 2, jp)]
                        nc.tensor.matmul(
                            ps[:rows, 64 * pl:64 * pl + 64],
                            kt_[:, kt_off:kt_off + rows], qbd[v][jp][:],
                            start=True, stop=True)
                    pl_t = sb.tile([128, 128], BF16, tag="plT", bufs=8,
                                   name="plT")
                    nc.scalar.activation(pl_t[:rows, :], ps[:rows, 0:128],
                                         AF.Exp, scale=SCALE)
                    epi_pl[(v, pp, c)] = pl_t
                return go

            def task_epi_av(v, pp, c):
                def go():
                    c0, rows, f2, ci = _chunk_geo(c)
                    pl_t = epi_pl.pop((v, pp, c))
                    vt_ = epi_vt.pop((v, pp, c))
                    nc.tensor.matmul(epi_acc[v][0:128, 0:4 * (D + 1)],
                                     pl_t[:rows, :], vt_[:rows, :],
                                     start=(c == 0), stop=(c == NKC - 1))
                return go

            def task_esum(v, pp):
                def go():
                    acc = epi_acc[v]
                    sg_ = sb.tile([128, 4], F32, tag="esum", bufs=2,
                                  name="esum")
                    nc.vector.tensor_copy(
                        sg_[:], acc[:, :].rearrange("p (h d) -> p h d",
                                                    h=4)[:, :, D:D + 1]
                        .rearrange("p h d -> p (h d)"))
                    rcs = sb.tile([128, 4], F32, tag="ercs", bufs=2,
                                  name="ercs")
                    nc.vector.reciprocal(rcs[:], sg_[:])
                    epi_rcs[v] = rcs
                return go

            def task_eext(v, pp, hl):
                def go():
                    acc = epi_acc[v]
                    rcs = epi_rcs[v]
                    hg = 4 * pp + hl
                    r0 = 64 * (hl // 2) + 32 * (hl % 2)
                    st = sb.tile([F, D], BF16, tag="st", bufs=4, name="st")
                    nc.vector.tensor_scalar_mul(
                        st[:], acc[r0:r0 + F, 65 * hl:65 * hl + D],
                        rcs[r0:r0 + F, hl:hl + 1])
                    pst = next_psc()[:].bitcast(BF16)
                    nc.tensor.transpose(pst[:D, 0:F], st[:], identh[:F, :F])
                    nc.scalar.copy(
                        ocfT[hg // 2][64 * (hg % 2):64 * (hg % 2) + D,
                                      F * v:F * (v + 1)],
                        pst[:D, 0:F])
                return go

            def epi_tasks(v):
                tasks = [task_qbd(v)]
                for pp in range(3):
                    for c in range(NKC):
                        tasks.append(task_epi_sc(v, pp, c))
                        if c >= 3:
                            tasks.append(task_epi_av(v, pp, c - 3))
                    for c in range(NKC - 3, NKC):
                        tasks.append(task_epi_av(v, pp, c))
                    tasks.append(task_esum(v, pp))
                    tasks += [task_eext(v, pp, hl) for hl in range(4)]
                return tasks

            # ---------------- finale: CLS adapter + cross-frame out-proj ----
            def emit_finale():
                yclsh = sb.tile([S, E], BF16, bufs=1, tag="yclsh",
                                name="yclsh")
                nc.vector.tensor_copy(yclsh[:], ycls[:])
                yclsT = []
                for k in range(KT):
                    pst = next_psc()[:].bitcast(BF16)
                    nc.tensor.transpose(pst[:, :S], yclsh[:, 128 * k:128 * (k + 1)],
                                        identh[:S, :S])
                    t_ = sb.tile([128, S], BF16, tag="yclsT", bufs=6,
                                 name=f"yclsT{k}")
                    nc.scalar.copy(t_[:], pst[:, :S])
                    yclsT.append(t_)
                ps8 = psav.tile([128, 512], F32, tag="pav", name="ps8")
                for k in range(KT):
                    nc.tensor.matmul(ps8[:R, :S], dwtt[:, R * k:R * (k + 1)],
                                     yclsT[k][:],
                                     start=(k == 0), stop=(k == KT - 1))
                z = sb.tile([R, S], F32, tag="z8", name="z8")
                if has_down_bias:
                    nc.scalar.activation(z[:], ps8[:R, :S], AF.Identity,
                                         bias=bdown[:, 0:1])
                else:
                    nc.scalar.copy(z[:], ps8[:R, :S])
                en = sb.tile([R, S], F32, tag="sg8", name="sg8")
                nc.scalar.activation(en[:], z[:], AF.Exp, scale=-1.702)
                nc.vector.tensor_scalar_add(en[:], en[:], 1.0)
                rec = sb.tile([R, S], F32, tag="rec8", name="rec8")
                nc.vector.reciprocal_approx_fast(rec[:], en[:])
                gq = sb.tile([R, S], BF16, tag="gq8", name="gq8")
                nc.vector.tensor_tensor(out=gq[:], in0=z[:], in1=rec[:],
                                        op=MUL)
                cn = sb.tile([S, E], F32, tag="cn", bufs=1, name="cn")
                for m in range(KT):
                    ps = psmm.tile([128, 512], F32, tag="pmm", name="psf")
                    nc.tensor.matmul(ps[:, :S], uwt[:, 128 * m:128 * (m + 1)],
                                     gq[:], start=True, stop=False)
                    for k in range(KT):
                        nc.tensor.matmul(ps[:, :S],
                                         wot[:, E * k + 128 * m:
                                              E * k + 128 * (m + 1)],
                                         ocfT[k][:], start=False,
                                         stop=(k == KT - 1))
                    cnT = sb.tile([128, S], F32, tag="cnT", bufs=2, name="cnT")
                    if has_cls_bias:
                        nc.scalar.activation(cnT[:], ps[:, :S], AF.Identity,
                                             bias=bcls[:, m:m + 1])
                    else:
                        if m % 2 == 0:
                            nc.scalar.copy(cnT[:], ps[:, :S])
                        else:
                            nc.vector.tensor_copy(cnT[:], ps[:, :S])
                    pst = next_psc()
                    nc.tensor.transpose(pst[:S, 0:128], cnT[:], identf[:, :])
                    if m % 2 == 0:
                        nc.vector.tensor_copy(cn[:, 128 * m:128 * (m + 1)],
                                              pst[:S, 0:128])
                    else:
                        nc.scalar.copy(cn[:, 128 * m:128 * (m + 1)],
                                       pst[:S, 0:128])
                nc.sync.dma_start(out=y_d[:, 0, :], in_=cn[:])

            # ---------------- emission schedule ----------------
            load_xt(2)
            epi0 = None
            ntt = (GT + 127) // 128
            for p in range(NP):
                g = (2 * p) // G
                if (2 * p) % G == 0:
                    attnTg[g] = [sb.tile([128, GT], BF16, tag="gt", bufs=12,
                                         name=f"gt{g}_{k}")
                                 for k in range(KT)]
                if p + 3 < NP:
                    load_xt(p + 3)
                emit_pair_qkv(p)
                emit_pair_v(p)
                if p == 11:
                    # tail: interleave the last pair's attention, the last
                    # group's out-proj, and the second video's epilogue so
                    # exp- and DMA-bound chains hide behind tensor-dense work
                    at = attn_tasks(11)
                    op_ = [task_group_tile(2, tt) for tt in range(ntt)]
                    ep = epi_tasks(1)
                    ia = ib = 0
                    while ia < len(at) or ib < len(ep):
                        if ia < len(at):
                            pend.append(at[ia]); ia += 1
                        for _ in range(2):
                            if ib < len(ep):
                                pend.append(ep[ib]); ib += 1
                        if ia >= len(at) and op_:
                            pend.append(op_.pop(0))
                    pend.extend(op_)
                else:
                    pend.extend(attn_tasks(p))
                    if p % 4 == 3:
                        for tt in range(ntt):
                            pend.append(task_group_tile(g, tt))
                if p == 5:
                    epi0 = epi_tasks(0)
                if epi0:
                    pend.extend(epi0[:45])
                    epi0 = epi0[45:]
                # bound the backlog: stale tasks past ~1 pair break the ring
                # buffers' reuse-distance assumptions
                while len(pend) > 36:
                    drain(1)
            drain(len(pend))
            emit_finale()

    nc.finalize()
    return nc


def _preprocess(x, in_proj_weight, in_proj_bias, out_proj_weight,
                out_proj_bias, lora_a, lora_b, down_w, down_b, up_w, up_b):
    w_comb = in_proj_weight.astype(np.float64) + \
        lora_b.astype(np.float64) @ lora_a.astype(np.float64)
    w_comb = w_comb.astype(np.float32)
    b_v = in_proj_bias[2 * E:3 * E].astype(np.float32)
    bias_row = out_proj_bias.astype(np.float32) + \
        b_v @ out_proj_weight.T.astype(np.float32)
    b_cls = bias_row + up_b.astype(np.float32)
    def _pack(w):  # [E, C] -> [128, KT*C] with k-major columns
        kt = w.reshape(KT, 128, -1)
        return _bf(kt.transpose(1, 0, 2).reshape(128, -1))

    p = {
        "w_qkt": _bf(np.ascontiguousarray(
            w_comb[0:2 * E].T.reshape(KT, 128, 2 * KT, 128)
            .transpose(2, 1, 0, 3).reshape(2 * KT, 128, KT * 128))),
        "w_vt": _pack(np.ascontiguousarray(w_comb[2 * E:3 * E].T)),
        "w_ot": _pack(np.ascontiguousarray(out_proj_weight.T)),
        "b_qk_t": _f32(in_proj_bias[0:2 * E].reshape(2 * KT, 128).T),
        "bias_row_o": _bf(bias_row.reshape(1, E)),
        "b_cls_t": _f32(b_cls.reshape(KT, 128).T),
        "down_wt": _pack(np.ascontiguousarray(down_w.T)),
        "b_down": _f32(down_b.reshape(R, 1)),
        "up_wt": _bf(up_w.T),
    }
    flags = (
        bool(np.any(in_proj_bias[0:2 * E])),
        bool(np.any(bias_row)),
        bool(np.any(down_b)),
        bool(np.any(b_cls)),
    )
    # xt per core: [NP, 128, KT*T2] with cols k-major, two seqs side by side
    xts = []
    xb = x.astype(ml_dtypes.bfloat16)
    for c in range(NCORES):
        xc = xb[S * c:S * (c + 1)]            # [S, T, E]
        xt = np.empty((NP, 128, KT * T2), dtype=ml_dtypes.bfloat16)
        for pi in range(NP):
            a = xc[2 * pi].T.reshape(KT, 128, T)      # [KT, 128, T]
            b = xc[2 * pi + 1].T.reshape(KT, 128, T)
            blk = np.concatenate([a, b], axis=2)      # [KT, 128, T2]
            xt[pi] = blk.transpose(1, 0, 2).reshape(128, KT * T2)
        xts.append(np.ascontiguousarray(xt))
    return p, flags, xts


def kernel(x, in_proj_weight, in_proj_bias, out_proj_weight, out_proj_bias,
           lora_a, lora_b, down_w, down_b, up_w, up_b,
           b, n_f, token_len, d_v):
    global _last_results
    x = np.asarray(x, dtype=np.float32)
    assert x.shape == (B * F, T, E), x.shape
    params, flags, xts = _preprocess(
        x, np.asarray(in_proj_weight), np.asarray(in_proj_bias),
        np.asarray(out_proj_weight), np.asarray(out_proj_bias),
        np.asarray(lora_a), np.asarray(lora_b),
        np.asarray(down_w), np.asarray(down_b),
        np.asarray(up_w), np.asarray(up_b))

    nc = _build(*flags)

    in_maps = []
    for c in range(NCORES):
        m = dict(params)
        m["xt"] = xts[c]
        in_maps.append(m)

    res = run_bass_kernel_spmd(nc, in_maps, list(range(NCORES)))
    _last_results = res
    out = np.concatenate([res.results[c]["y"] for c in range(NCORES)], axis=0)
    return out.astype(np.float32)
